# revision 30
# baseline (speedup 1.0000x reference)
"""Dense GAT (2-layer, 8+1 heads) on 8 Trainium2 NeuronCores — V3.

Row-parallel over destination rows i (R=512 per core). Per core:
  - adjacency arrives HOST-TRANSPOSED as adjT[j, i] in {0, 65504} fp16
    (mask applied via tensor MIN, no PE transposes needed).
  - h1|a_src from one fp16 matmul chain against host-folded
    [w1 | w1@blockdiag(att_src1)]; fp16 PSUM.
  - scaled-attention trick: softmax over j is invariant to any per-i
    factor, so E is normalized by exp(0.2*ad_i):
        e~[j,i] = min(adjT[j,i], max(eas_j * u8[i], e2as_j))
    with u8 = exp(0.8*ad_i) broadcast (one per head), eas = exp(as_j),
    e2as = exp(0.2*as_j) per-partition scalars.
    DVE chain: one tensor_scalar (4x mode) + mask-min (quad-batched).
    ACT chain (some tiles): Relu(u8*eas - e2as) + Identity(r + e2as).
    Mask-min on DVE or GPSIMD (Pool) per static schedule.
  - softmax denominators ride as a ones column in the aggregation lhsT.
  - one AllGather of [512, 66] f32 (h2+b2 packed fp16 | eas2 | e2as2).
  - L2 output normalized via per-partition ACT scale after PE transpose.
"""
import numpy as np

N = 4096
F_IN = 256
HID = 64
H1 = 8
F1 = H1 * HID
OUT = 128
N_CORES = 8
R = N // N_CORES
JT = N // 128          # 32 j-tiles
IT = R // 128          # 4 i-tiles
QT = JT // 4           # 8 quads of 4 j-tiles
NEG_ATT = 0.2
NEG_OUT = 0.01
MASKV = 65504.0        # fp16 max: adjacency "1" value; mask via min()

G = HID + 2            # bounce cols: 64 f32 words (128 f16 h2) | eas2 | e2as2

_CACHE = {}

# ---- static engine schedule knobs ----
# ACT chain quads: (h, qt) pairs routed to the scalar engine (2 ACT ops/tile)
ACT_QUADS = 22         # of 64 L1 quads on the ACT chain


def _is_act_quad(h, qt):
    return (h * QT + qt) * 7 % 64 < ACT_QUADS


def _build():
    import concourse.bass as bass
    from concourse import bacc
    import concourse.mybir as mybir
    import concourse.tile as tile
    from concourse.masks import make_identity

    f32 = mybir.dt.float32
    f16 = mybir.dt.float16
    A = mybir.ActivationFunctionType
    Al = mybir.AluOpType

    nc = bacc.Bacc("TRN2", target_bir_lowering=False, debug=False,
                   num_devices=N_CORES)
    d_xT16 = nc.dram_tensor("xT16", [F_IN, N], f16, kind="ExternalInput")
    d_xmT = nc.dram_tensor("xmT", [F_IN, R], f16, kind="ExternalInput")
    d_adjT = nc.dram_tensor("adjT", [N, R], f16, kind="ExternalInput")
    d_rhs1 = nc.dram_tensor("rhs1", [F_IN, F1 + H1], f16, kind="ExternalInput")
    d_vdst1 = nc.dram_tensor("vdst1", [F_IN, H1], f16, kind="ExternalInput")
    d_rhs2 = nc.dram_tensor("rhs2", [F1, OUT + 2], f16, kind="ExternalInput")
    d_b1c = nc.dram_tensor("b1c", [HID, H1], f32, kind="ExternalInput")
    d_b2r = nc.dram_tensor("b2r", [1, OUT + 2], f16, kind="ExternalInput")
    d_out = nc.dram_tensor("outR", [R, OUT], f32, kind="ExternalOutput")

    with tile.TileContext(nc) as tc:
        with tc.tile_pool(name="const", bufs=1) as const, \
             tc.tile_pool(name="big", bufs=1) as big, \
             tc.tile_pool(name="work", bufs=3) as work, \
             tc.tile_pool(name="qpool", bufs=3) as qpool, \
             tc.tile_pool(name="epool", bufs=3) as epool, \
             tc.tile_pool(name="dram", bufs=1, space="DRAM") as dram, \
             tc.tile_pool(name="ps_mm", bufs=2, space="PSUM") as ps_mm, \
             tc.tile_pool(name="ps_bc", bufs=2, space="PSUM") as ps_bc, \
             tc.tile_pool(name="ps_ag", bufs=2, space="PSUM") as ps_ag, \
             tc.tile_pool(name="ps_sm", bufs=2, space="PSUM") as ps_sm:
            ident = const.tile([128, 128], f32)
            make_identity(nc, ident)
            ident16 = const.tile([128, 128], f16)
            nc.vector.tensor_copy(ident16, ident)
            ones16 = const.tile([1, 128], f16)
            nc.vector.memset(ones16, 1.0)
            ones_col16 = const.tile([128, 1], f16)
            nc.vector.memset(ones_col16, 1.0)
            rhs1_sb = const.tile([128, 2, F1 + H1], f16)
            nc.sync.dma_start(out=rhs1_sb[:, 0, :], in_=d_rhs1[0:128, :])
            nc.sync.dma_start(out=rhs1_sb[:, 1, :], in_=d_rhs1[128:256, :])
            vdst1_sb = const.tile([128, 2, H1], f16)
            nc.sync.dma_start(out=vdst1_sb[:, 0, :], in_=d_vdst1[0:128, :])
            nc.sync.dma_start(out=vdst1_sb[:, 1, :], in_=d_vdst1[128:256, :])
            rhs2_sb = const.tile([128, 4, OUT + 2], f16)
            for kt in range(4):
                nc.sync.dma_start(out=rhs2_sb[:, kt, :],
                                  in_=d_rhs2[kt * 128:(kt + 1) * 128, :])
            b1_sb = const.tile([HID, H1], f32)
            nc.sync.dma_start(out=b1_sb, in_=d_b1c[:, :])
            b2r_sb = const.tile([1, OUT + 2], f16)
            nc.sync.dma_start(out=b2r_sb, in_=d_b2r[:, :])

            # ---- big persistent arrays ----
            adjT_all = big.tile([128, JT, R], f16)       # 32 KB/part
            xT_sb = big.tile([128, 2, N], f16)           # 16 KB/part
            h1_all = big.tile([128, JT, H1, HID + 1], f16)  # 32.5 KB/part
            asrc16 = big.tile([128, JT, H1], f16)
            easrc = big.tile([128, JT, H1], f32)
            e2src = big.tile([128, JT, H1], f32)
            nege2 = big.tile([128, JT, H1], f32)
            adstT = big.tile([H1, R], f32)
            adst_rows = big.tile([1, H1, R], f32)
            adst2T = big.tile([1, R], f32)
            x2T_all = big.tile([128, 4, R], f16)
            u8bc = big.tile([128, 2, R], f16)            # 2-head pipeline
            h2g_all = big.tile([128, N_CORES, IT, G], f32)
            u8bc2 = big.tile([128, R], f16)
            rz2col = big.tile([128, IT], f32)

            nc.vector.memset(h1_all[:, :, :, HID:HID + 1], 1.0)

            # ---- input DMAs: xmT/xT first (a_dst + h1 need them), adjT after
            xmT_sb = big.tile([128, 2, R], f16)
            nc.sync.dma_start(out=xmT_sb[:, 0, :], in_=d_xmT[0:128, :])
            nc.sync.dma_start(out=xmT_sb[:, 1, :], in_=d_xmT[128:256, :])
            for c4 in range(4):
                csl = slice(c4 * (N // 4), (c4 + 1) * (N // 4))
                nc.sync.dma_start(out=xT_sb[:, 0, csl], in_=d_xT16[0:128, csl])
                nc.sync.dma_start(out=xT_sb[:, 1, csl],
                                  in_=d_xT16[128:256, csl])
            for jb in range(JT):
                nc.sync.dma_start(
                    out=adjT_all[:, jb, :],
                    in_=d_adjT[jb * 128:(jb + 1) * 128, :])

            # ---- a_dst (own rows) -> adstT [H1, R] f32 ----
            for it in range(IT):
                ps_ad = ps_sm.tile([128, 128], f32, tag="sm")
                for kb in range(2):
                    nc.tensor.matmul(ps_ad[:, 0:H1],
                                     xmT_sb[:, kb, it * 128:(it + 1) * 128],
                                     vdst1_sb[:, kb, :],
                                     start=(kb == 0), stop=(kb == 1))
                adm = work.tile([128, H1], f32, tag="adm", bufs=2)
                nc.vector.tensor_copy(adm, ps_ad[:, 0:H1])
                ps_adT = ps_sm.tile([128, 128], f32, tag="sm")
                nc.tensor.transpose(ps_adT[0:H1, :], adm, ident)
                nc.vector.tensor_copy(adstT[:, it * 128:(it + 1) * 128],
                                      ps_adT[0:H1, :])
            for h in range(H1):
                nc.sync.dma_start(out=adst_rows[:, h, :], in_=adstT[h:h + 1, :])

            # ---- h1 | a_src per jt ----
            for jt in range(JT):
                cols = slice(jt * 128, (jt + 1) * 128)
                ps_h = ps_mm.tile([128, F1], f32, tag="h")
                ps_ast = ps_sm.tile([128, 128], f32, tag="sm")
                ps_as = ps_ast[:, 0:H1]
                for kb in range(2):
                    nc.tensor.matmul(ps_h, xT_sb[:, kb, cols],
                                     rhs1_sb[:, kb, 0:F1],
                                     start=(kb == 0), stop=(kb == 1))
                    nc.tensor.matmul(ps_as, xT_sb[:, kb, cols],
                                     rhs1_sb[:, kb, F1:F1 + H1],
                                     start=(kb == 0), stop=(kb == 1))
                # h1 [j, h, c] copies: 1 in 4 on DVE, rest on ACT
                if jt % 4 == 0:
                    nc.vector.tensor_copy(
                        h1_all[:, jt, :, 0:HID],
                        ps_h.rearrange("p (h c) -> p h c", c=HID))
                else:
                    nc.scalar.copy(
                        h1_all[:, jt, :, 0:HID],
                        ps_h.rearrange("p (h c) -> p h c", c=HID))
                nc.scalar.copy(asrc16[:, jt, :], ps_as)
                if jt % 8 == 7:
                    gs = slice(jt - 7, jt + 1)
                    nc.scalar.activation(easrc[:, gs, :], asrc16[:, gs, :],
                                         A.Exp)
                    nc.scalar.activation(e2src[:, gs, :], asrc16[:, gs, :],
                                         A.Exp, scale=NEG_ATT)
                    nc.scalar.activation(nege2[:, gs, :], e2src[:, gs, :],
                                         A.Identity, scale=-1.0)

            # ---- layer-1 attention, head-pipelined ----
            def _pre_head(h):
                u8row = work.tile([1, R], f16, tag="u8row", bufs=2,
                                  name=f"u8r{h}")
                nc.scalar.activation(u8row, adst_rows[:, h, :], A.Exp,
                                     scale=1.0 - NEG_ATT)
                ps_u8 = ps_bc.tile([128, R], f32, tag="bc", name=f"psu8{h}")
                nc.tensor.matmul(ps_u8, ones16, u8row, start=True, stop=True)
                nc.vector.tensor_copy(u8bc[:, h % 2, :], ps_u8)

            _pre_head(0)
            for h in range(H1):
                if h + 1 < H1:
                    _pre_head(h + 1)
                u8 = u8bc[:, h % 2, :]
                ps_agg = ps_ag.tile([HID + 1, R], f32, tag="agg")
                act_qts = [qt for qt in range(QT) if _is_act_quad(h, qt)]
                dve_qts = [qt for qt in range(QT) if not _is_act_quad(h, qt)]
                # ACT-chain q tiles first so the scalar engine runs ahead
                act_q4 = {}
                for qt in act_qts:
                    q4 = qpool.tile([128, 4, R], f16, tag="qa", bufs=3,
                                    name=f"qa{h}_{qt}")
                    for k in range(4):
                        jt = qt * 4 + k
                        r = work.tile([128, R], f16, tag="ract", bufs=3)
                        nc.scalar.activation(
                            r, u8, A.Relu,
                            bias=nege2[:, jt, h:h + 1],
                            scale=easrc[:, jt, h:h + 1])
                        nc.scalar.activation(
                            q4[:, k, :], r, A.Identity,
                            bias=e2src[:, jt, h:h + 1])
                    act_q4[qt] = q4
                n_mm = 0
                for qt in dve_qts + act_qts:
                    if qt in act_q4:
                        q4 = act_q4[qt]
                    else:
                        q4 = qpool.tile([128, 4, R], f16, tag="q")
                        for k in range(4):
                            jt = qt * 4 + k
                            nc.vector.tensor_scalar(
                                q4[:, k, :], u8,
                                easrc[:, jt, h:h + 1],
                                e2src[:, jt, h:h + 1],
                                op0=Al.mult, op1=Al.max)
                    e4 = epool.tile([128, 4, R], f16, tag="e")
                    nc.vector.tensor_tensor(
                        e4.rearrange("p a b -> p (a b)"),
                        q4.rearrange("p a b -> p (a b)"),
                        adjT_all[:, qt * 4:(qt + 1) * 4, :].rearrange(
                            "p a b -> p (a b)"),
                        op=Al.min)
                    for k in range(4):
                        jt = qt * 4 + k
                        nc.tensor.matmul(ps_agg, h1_all[:, jt, h, :],
                                         e4[:, k, :],
                                         start=(n_mm == 0),
                                         stop=(n_mm == JT - 1))
                        n_mm += 1
                rz = work.tile([1, R], f16, tag="rz", bufs=2)
                with nc.allow_low_precision(reason="1/z in fp16: 1e-3 rel ok"):
                    nc.vector.reciprocal(rz, ps_agg[HID:HID + 1, :])
                ps_rzb = ps_bc.tile([128, R], f32, tag="bc")
                nc.tensor.matmul(ps_rzb[0:HID, :], ones16[:, 0:HID], rz,
                                 start=True, stop=True)
                rzb_sb = work.tile([HID, R], f16, tag="rzb", bufs=2)
                nc.scalar.copy(rzb_sb, ps_rzb[0:HID, :])
                y_h = work.tile([HID, R], f16, tag="yh", bufs=2)
                nc.vector.tensor_mul(y_h, ps_agg[0:HID, :], rzb_sb)
                po = (h % 2) * HID
                nc.scalar.activation(
                    x2T_all[po:po + HID, h // 2, :], y_h, A.Prelu,
                    bias=b1_sb[:, h:h + 1], alpha=NEG_OUT)

            # ---- layer 2: h2 per it, bounce, single AllGather ----
            bounce_in = dram.tile([R, G], f32, name="bin")
            bounce_out = dram.tile([N_CORES, R, G], f32,
                                   addr_space="Shared", name="bout")
            for it in range(IT):
                ps_h2t = ps_mm.tile([128, R], f32, tag="h")
                ps_h2 = ps_h2t[:, 0:OUT + 2]
                for kt in range(4):
                    nc.tensor.matmul(
                        ps_h2,
                        x2T_all[:, kt, it * 128:(it + 1) * 128],
                        rhs2_sb[:, kt, :],
                        start=(kt == 0), stop=False)
                nc.tensor.matmul(ps_h2, ones16, b2r_sb,
                                 start=False, stop=True)
                h2m = work.tile([128, G], f32, tag="h2m", bufs=2)
                nc.vector.tensor_copy(h2m[:, 0:HID].bitcast(f16),
                                      ps_h2[:, 0:OUT])
                nc.scalar.activation(h2m[:, HID:HID + 1],
                                     ps_h2[:, OUT:OUT + 1], A.Exp)
                nc.scalar.activation(h2m[:, HID + 1:HID + 2],
                                     ps_h2[:, OUT:OUT + 1], A.Exp,
                                     scale=NEG_ATT)
                nc.sync.dma_start(
                    out=bounce_in[it * 128:(it + 1) * 128, :], in_=h2m)
                ad2m = work.tile([128, 1], f32, tag="ad2m", bufs=2)
                nc.scalar.copy(ad2m, ps_h2[:, OUT + 1:OUT + 2])
                ps_adT2 = ps_sm.tile([1, 128], f32, tag="sm")
                nc.tensor.transpose(ps_adT2, ad2m, ident)
                nc.vector.tensor_copy(adst2T[:, it * 128:(it + 1) * 128],
                                      ps_adT2)
            nc.gpsimd.collective_compute(
                "AllGather",
                bass.mybir.AluOpType.bypass,
                replica_groups=[list(range(N_CORES))],
                ins=[bounce_in.opt()],
                outs=[bounce_out.opt()],
            )
            for c8 in range(N_CORES):
                nc.sync.dma_start(
                    out=h2g_all[:, c8, :, :],
                    in_=bounce_out[c8].rearrange("(r1 p) g -> p r1 g", p=128))

            # ---- layer-2 attention ----
            u8row2 = work.tile([1, R], f16, tag="u8row", bufs=2)
            nc.scalar.activation(u8row2, adst2T, A.Exp, scale=1.0 - NEG_ATT)
            ps_u82 = ps_bc.tile([128, R], f32, tag="bc")
            nc.tensor.matmul(ps_u82, ones16, u8row2, start=True, stop=True)
            nc.vector.tensor_copy(u8bc2, ps_u82)

            # reuse L1 pools: o2 in ps_mm "h" shape, z2 rides an "agg" buffer
            ps_o2 = ps_mm.tile([128, R], f32, tag="h")
            ps_z2t = ps_ag.tile([HID + 1, R], f32, tag="agg")
            ps_z2 = ps_z2t[HID:HID + 1, :]
            for qt in range(QT):
                q4 = qpool.tile([128, 4, R], f16, tag="q")
                for k in range(4):
                    jt = qt * 4 + k
                    c8, r1 = jt // IT, jt % IT
                    nc.vector.tensor_scalar(
                        q4[:, k, :], u8bc2,
                        h2g_all[:, c8, r1, HID:HID + 1],
                        h2g_all[:, c8, r1, HID + 1:HID + 2],
                        op0=Al.mult, op1=Al.max)
                e4 = epool.tile([128, 4, R], f16, tag="e")
                nc.vector.tensor_tensor(
                    e4.rearrange("p a b -> p (a b)"),
                    q4.rearrange("p a b -> p (a b)"),
                    adjT_all[:, qt * 4:(qt + 1) * 4, :].rearrange(
                        "p a b -> p (a b)"),
                    op=Al.min)
                for k in range(4):
                    jt = qt * 4 + k
                    c8, r1 = jt // IT, jt % IT
                    nc.tensor.matmul(
                        ps_o2, h2g_all[:, c8, r1, 0:HID].bitcast(f16),
                        e4[:, k, :],
                        start=(jt == 0), stop=(jt == JT - 1))
                    nc.tensor.matmul(
                        ps_z2, ones_col16, e4[:, k, :],
                        start=(jt == 0), stop=(jt == JT - 1))
            # per-it: transpose z2 chunk -> recip col; transpose o2 -> prelu
            o2sb = work.tile([128, R], f32, tag="o2sb", bufs=1)
            for it in range(IT):
                nc.scalar.copy(o2sb[:, it * 128:(it + 1) * 128],
                               ps_o2[:, it * 128:(it + 1) * 128])
            z2sb = work.tile([1, R], f16, tag="z2sb", bufs=1)
            nc.vector.tensor_copy(z2sb, ps_z2)
            del ps_z2t
            outT_sb = work.tile([128, IT, OUT], f32, tag="outT", bufs=1)
            for it in range(IT):
                isl = slice(it * 128, (it + 1) * 128)
                ps_zTt = ps_sm.tile([128, 128], f32, tag="sm")
                ps_zT = ps_zTt.bitcast(f16)[:, 0:1]
                nc.tensor.transpose(ps_zT, z2sb[:, isl], ident16[0:1, 0:1])
                with nc.allow_low_precision(reason="1/z2 col fp16 src ok"):
                    nc.vector.reciprocal(rz2col[:, it:it + 1], ps_zT)
                ps_oT = ps_sm.tile([128, 128], f32, tag="sm")
                nc.tensor.transpose(ps_oT, o2sb[:, isl], ident)
                nc.scalar.activation(outT_sb[:, it, :], ps_oT, A.Prelu,
                                     scale=rz2col[:, it:it + 1], alpha=NEG_OUT)
            nc.sync.dma_start(
                out=d_out.rearrange("(i p) c -> p i c", p=128), in_=outT_sb)

    nc.finalize()
    return nc


def _prep_host(x, adj, w1, att_src1, att_dst1, b1, w2, att_src2, att_dst2, b2):
    x = np.asarray(x, np.float32).reshape(N, F_IN)
    adj = np.asarray(adj, np.float32).reshape(N, N)
    w1 = np.asarray(w1, np.float32)
    w2 = np.asarray(w2, np.float32)
    att_src1 = np.asarray(att_src1, np.float32)
    att_dst1 = np.asarray(att_dst1, np.float32)
    att_src2 = np.asarray(att_src2, np.float32)
    att_dst2 = np.asarray(att_dst2, np.float32)
    b1 = np.asarray(b1, np.float32)
    b2 = np.asarray(b2, np.float32)

    xT = np.ascontiguousarray(x.T)
    xT16 = xT.astype(np.float16)
    adjm = (adj * MASKV).astype(np.float16)
    v_src1 = np.empty((F_IN, H1), np.float32)
    v_dst1 = np.empty((F_IN, H1), np.float32)
    for h in range(H1):
        blk = w1[:, h * HID:(h + 1) * HID]
        v_src1[:, h] = blk @ att_src1[h]
        v_dst1[:, h] = blk @ att_dst1[h]
    rhs1 = np.ascontiguousarray(
        np.concatenate([w1, v_src1], axis=1)).astype(np.float16)
    v_src2 = (w2 @ att_src2[0])[:, None]
    v_dst2 = (w2 @ att_dst2[0])[:, None]
    rhs2 = np.ascontiguousarray(
        np.concatenate([w2, v_src2, v_dst2], axis=1)).astype(np.float16)
    b1c = np.ascontiguousarray(b1.reshape(H1, HID).T)
    b2r = np.zeros((1, OUT + 2), np.float16)
    b2r[0, 0:OUT] = b2

    in_maps = []
    for c in range(N_CORES):
        rows = slice(c * R, (c + 1) * R)
        in_maps.append({
            "xT16": xT16,
            "xmT": np.ascontiguousarray(xT16[:, rows]),
            "adjT": np.ascontiguousarray(adjm[rows, :].T),
            "rhs1": rhs1,
            "vdst1": v_dst1.astype(np.float16),
            "rhs2": rhs2,
            "b1c": b1c,
            "b2r": b2r,
        })
    return in_maps


def kernel(**inputs) -> np.ndarray:
    from concourse.bass_utils import run_bass_kernel_spmd

    if "nc" not in _CACHE:
        _CACHE["nc"] = _build()
    nc = _CACHE["nc"]
    in_maps = _prep_host(**inputs)
    try:
        res = run_bass_kernel_spmd(nc, in_maps, list(range(N_CORES)))
    except Exception:
        # transient NRT device wedge — one clean retry
        res = run_bass_kernel_spmd(nc, in_maps, list(range(N_CORES)))
    out = np.empty((1, N, OUT), np.float32)
    for c in range(N_CORES):
        out[0, c * R:(c + 1) * R, :] = res.results[c]["outR"]
    return out


# revision 32
# speedup vs baseline: 1.0637x; 1.0637x over previous
"""Dense GAT (2-layer, 8+1 heads) on 8 Trainium2 NeuronCores — V3.

Row-parallel over destination rows i (R=512 per core). Per core:
  - adjacency arrives HOST-TRANSPOSED as adjT[j, i] in {0, 65504} fp16
    (mask applied via tensor MIN, no PE transposes needed).
  - h1|a_src from one fp16 matmul chain against host-folded
    [w1 | w1@blockdiag(att_src1)]; fp16 PSUM.
  - scaled-attention trick: softmax over j is invariant to any per-i
    factor, so E is normalized by exp(0.2*ad_i):
        e~[j,i] = min(adjT[j,i], max(eas_j * u8[i], e2as_j))
    with u8 = exp(0.8*ad_i) broadcast (one per head), eas = exp(as_j),
    e2as = exp(0.2*as_j) per-partition scalars.
    DVE chain: one tensor_scalar (4x mode) + mask-min (quad-batched).
    ACT chain (some tiles): Relu(u8*eas - e2as) + Identity(r + e2as).
    Mask-min on DVE or GPSIMD (Pool) per static schedule.
  - softmax denominators ride as a ones column in the aggregation lhsT.
  - one AllGather of [512, 66] f32 (h2+b2 packed fp16 | eas2 | e2as2).
  - L2 output normalized via per-partition ACT scale after PE transpose.
"""
import numpy as np

N = 4096
F_IN = 256
HID = 64
H1 = 8
F1 = H1 * HID
OUT = 128
N_CORES = 8
R = N // N_CORES
JT = N // 128          # 32 j-tiles
IT = R // 128          # 4 i-tiles
QT = JT // 4           # 8 quads of 4 j-tiles
NEG_ATT = 0.2
NEG_OUT = 0.01
MASKV = 65504.0        # fp16 max: adjacency "1" value; mask via min()

G = HID + 2            # bounce cols: 64 f32 words (128 f16 h2) | eas2 | e2as2

_CACHE = {}

# ---- static engine schedule knobs ----
# ACT chain quads: (h, qt) pairs routed to the scalar engine (2 ACT ops/tile)
ACT_QUADS = 18         # of 64 L1 quads on the ACT chain
_H1MOD = 2


def _is_act_quad(h, qt):
    return (h * QT + qt) * 7 % 64 < ACT_QUADS


def _build():
    import concourse.bass as bass
    from concourse import bacc
    import concourse.mybir as mybir
    import concourse.tile as tile
    from concourse.masks import make_identity

    f32 = mybir.dt.float32
    f16 = mybir.dt.float16
    A = mybir.ActivationFunctionType
    Al = mybir.AluOpType

    nc = bacc.Bacc("TRN2", target_bir_lowering=False, debug=False,
                   num_devices=N_CORES)
    d_xT16 = nc.dram_tensor("xT16", [F_IN, N], f16, kind="ExternalInput")
    d_xmT = nc.dram_tensor("xmT", [F_IN, R], f16, kind="ExternalInput")
    d_adjT = nc.dram_tensor("adjT", [N, R], f16, kind="ExternalInput")
    d_rhs1 = nc.dram_tensor("rhs1", [F_IN, F1 + H1], f16, kind="ExternalInput")
    d_vdst1 = nc.dram_tensor("vdst1", [F_IN, H1], f16, kind="ExternalInput")
    d_rhs2 = nc.dram_tensor("rhs2", [F1, OUT + 2], f16, kind="ExternalInput")
    d_b1c = nc.dram_tensor("b1c", [HID, H1], f32, kind="ExternalInput")
    d_b2r = nc.dram_tensor("b2r", [1, OUT + 2], f16, kind="ExternalInput")
    d_out = nc.dram_tensor("outR", [R, OUT], f32, kind="ExternalOutput")

    with tile.TileContext(nc) as tc:
        with tc.tile_pool(name="const", bufs=1) as const, \
             tc.tile_pool(name="big", bufs=1) as big, \
             tc.tile_pool(name="work", bufs=3) as work, \
             tc.tile_pool(name="qpool", bufs=3) as qpool, \
             tc.tile_pool(name="epool", bufs=3) as epool, \
             tc.tile_pool(name="dram", bufs=1, space="DRAM") as dram, \
             tc.tile_pool(name="ps_mm", bufs=2, space="PSUM") as ps_mm, \
             tc.tile_pool(name="ps_bc", bufs=2, space="PSUM") as ps_bc, \
             tc.tile_pool(name="ps_ag", bufs=2, space="PSUM") as ps_ag, \
             tc.tile_pool(name="ps_sm", bufs=2, space="PSUM") as ps_sm:
            ident = const.tile([128, 128], f32)
            make_identity(nc, ident)
            ident16 = const.tile([128, 128], f16)
            nc.vector.tensor_copy(ident16, ident)
            ones16 = const.tile([1, 128], f16)
            nc.vector.memset(ones16, 1.0)
            ones_col16 = const.tile([128, 1], f16)
            nc.vector.memset(ones_col16, 1.0)
            rhs1_sb = const.tile([128, 2, F1 + H1], f16)
            nc.sync.dma_start(out=rhs1_sb[:, 0, :], in_=d_rhs1[0:128, :])
            nc.sync.dma_start(out=rhs1_sb[:, 1, :], in_=d_rhs1[128:256, :])
            vdst1_sb = const.tile([128, 2, H1], f16)
            nc.sync.dma_start(out=vdst1_sb[:, 0, :], in_=d_vdst1[0:128, :])
            nc.sync.dma_start(out=vdst1_sb[:, 1, :], in_=d_vdst1[128:256, :])
            rhs2_sb = const.tile([128, 4, OUT + 2], f16)
            for kt in range(4):
                nc.sync.dma_start(out=rhs2_sb[:, kt, :],
                                  in_=d_rhs2[kt * 128:(kt + 1) * 128, :])
            b1_sb = const.tile([HID, H1], f32)
            nc.sync.dma_start(out=b1_sb, in_=d_b1c[:, :])
            b2r_sb = const.tile([1, OUT + 2], f16)
            nc.sync.dma_start(out=b2r_sb, in_=d_b2r[:, :])

            # ---- big persistent arrays ----
            adjT_all = big.tile([128, JT, R], f16)       # 32 KB/part
            xT_sb = big.tile([128, 2, N], f16)           # 16 KB/part
            h1_all = big.tile([128, JT, H1, HID + 1], f16)  # 32.5 KB/part
            asrc16 = big.tile([128, JT, H1], f16)
            easrc = big.tile([128, JT, H1], f32)
            e2src = big.tile([128, JT, H1], f32)
            nege2 = big.tile([128, JT, H1], f32)
            adstT = big.tile([H1, R], f32)
            adst_rows = big.tile([1, H1, R], f32)
            adst2T = big.tile([1, R], f32)
            x2T_all = big.tile([128, 4, R], f16)
            u8bc = big.tile([128, 2, R], f16)            # 2-head pipeline
            h2g_all = big.tile([128, N_CORES, IT, G], f32)
            u8bc2 = big.tile([128, R], f16)
            rz2col = big.tile([128, IT], f32)

            nc.vector.memset(h1_all[:, :, :, HID:HID + 1], 1.0)

            # ---- input DMAs: xmT/xT first (a_dst + h1 need them), adjT after
            xmT_sb = big.tile([128, 2, R], f16)
            nc.sync.dma_start(out=xmT_sb[:, 0, :], in_=d_xmT[0:128, :])
            nc.sync.dma_start(out=xmT_sb[:, 1, :], in_=d_xmT[128:256, :])
            for c4 in range(4):
                csl = slice(c4 * (N // 4), (c4 + 1) * (N // 4))
                nc.sync.dma_start(out=xT_sb[:, 0, csl], in_=d_xT16[0:128, csl])
                nc.sync.dma_start(out=xT_sb[:, 1, csl],
                                  in_=d_xT16[128:256, csl])
            for jb in range(8):
                nc.sync.dma_start(
                    out=adjT_all[:, jb, :],
                    in_=d_adjT[jb * 128:(jb + 1) * 128, :])

            # ---- a_dst (own rows) -> adstT [H1, R] f32 ----
            for it in range(IT):
                ps_ad = ps_sm.tile([128, 128], f32, tag="sm")
                for kb in range(2):
                    nc.tensor.matmul(ps_ad[:, 0:H1],
                                     xmT_sb[:, kb, it * 128:(it + 1) * 128],
                                     vdst1_sb[:, kb, :],
                                     start=(kb == 0), stop=(kb == 1))
                adm = work.tile([128, H1], f32, tag="adm", bufs=2)
                nc.vector.tensor_copy(adm, ps_ad[:, 0:H1])
                ps_adT = ps_sm.tile([128, 128], f32, tag="sm")
                nc.tensor.transpose(ps_adT[0:H1, :], adm, ident)
                nc.vector.tensor_copy(adstT[:, it * 128:(it + 1) * 128],
                                      ps_adT[0:H1, :])
            for h in range(H1):
                nc.sync.dma_start(out=adst_rows[:, h, :], in_=adstT[h:h + 1, :])
            for jb in range(8, JT):
                nc.sync.dma_start(
                    out=adjT_all[:, jb, :],
                    in_=d_adjT[jb * 128:(jb + 1) * 128, :])

            # ---- h1 | a_src per jt ----
            for jt in range(JT):
                cols = slice(jt * 128, (jt + 1) * 128)
                ps_h = ps_mm.tile([128, F1], f32, tag="h")
                ps_ast = ps_sm.tile([128, 128], f32, tag="sm")
                ps_as = ps_ast[:, 0:H1]
                for kb in range(2):
                    nc.tensor.matmul(ps_h, xT_sb[:, kb, cols],
                                     rhs1_sb[:, kb, 0:F1],
                                     start=(kb == 0), stop=(kb == 1))
                    nc.tensor.matmul(ps_as, xT_sb[:, kb, cols],
                                     rhs1_sb[:, kb, F1:F1 + H1],
                                     start=(kb == 0), stop=(kb == 1))
                # h1 [j, h, c] copies: 1 in _H1MOD on DVE, rest on ACT
                if jt % _H1MOD == 0:
                    nc.vector.tensor_copy(
                        h1_all[:, jt, :, 0:HID],
                        ps_h.rearrange("p (h c) -> p h c", c=HID))
                else:
                    nc.scalar.copy(
                        h1_all[:, jt, :, 0:HID],
                        ps_h.rearrange("p (h c) -> p h c", c=HID))
                nc.scalar.copy(asrc16[:, jt, :], ps_as)
                if jt % 8 == 7:
                    gs = slice(jt - 7, jt + 1)
                    nc.scalar.activation(easrc[:, gs, :], asrc16[:, gs, :],
                                         A.Exp)
                    nc.scalar.activation(e2src[:, gs, :], asrc16[:, gs, :],
                                         A.Exp, scale=NEG_ATT)
                    nc.scalar.activation(nege2[:, gs, :], e2src[:, gs, :],
                                         A.Identity, scale=-1.0)

            # ---- layer-1 attention, head-pipelined ----
            def _pre_head(h):
                u8row = work.tile([1, R], f16, tag="u8row", bufs=2,
                                  name=f"u8r{h}")
                nc.scalar.activation(u8row, adst_rows[:, h, :], A.Exp,
                                     scale=1.0 - NEG_ATT)
                ps_u8 = ps_bc.tile([128, R], f32, tag="bc", name=f"psu8{h}")
                nc.tensor.matmul(ps_u8, ones16, u8row, start=True, stop=True)
                nc.vector.tensor_copy(u8bc[:, h % 2, :], ps_u8)

            _pre_head(0)
            for h in range(H1):
                if h + 1 < H1:
                    _pre_head(h + 1)
                u8 = u8bc[:, h % 2, :]
                ps_agg = ps_ag.tile([HID + 1, R], f32, tag="agg")
                act_qts = [qt for qt in range(QT) if _is_act_quad(h, qt)]
                dve_qts = [qt for qt in range(QT) if not _is_act_quad(h, qt)]
                # ACT-chain q tiles first so the scalar engine runs ahead
                act_q4 = {}
                for qt in act_qts:
                    q4 = qpool.tile([128, 4, R], f16, tag="qa", bufs=3,
                                    name=f"qa{h}_{qt}")
                    for k in range(4):
                        jt = qt * 4 + k
                        r = work.tile([128, R], f16, tag="ract", bufs=3)
                        nc.scalar.activation(
                            r, u8, A.Relu,
                            bias=nege2[:, jt, h:h + 1],
                            scale=easrc[:, jt, h:h + 1])
                        nc.scalar.activation(
                            q4[:, k, :], r, A.Identity,
                            bias=e2src[:, jt, h:h + 1])
                    act_q4[qt] = q4
                n_mm = 0
                for qt in dve_qts + act_qts:
                    if qt in act_q4:
                        q4 = act_q4[qt]
                    else:
                        q4 = qpool.tile([128, 4, R], f16, tag="q")
                        for k in range(4):
                            jt = qt * 4 + k
                            nc.vector.tensor_scalar(
                                q4[:, k, :], u8,
                                easrc[:, jt, h:h + 1],
                                e2src[:, jt, h:h + 1],
                                op0=Al.mult, op1=Al.max)
                    e4 = epool.tile([128, 4, R], f16, tag="e")
                    nc.vector.tensor_tensor(
                        e4.rearrange("p a b -> p (a b)"),
                        q4.rearrange("p a b -> p (a b)"),
                        adjT_all[:, qt * 4:(qt + 1) * 4, :].rearrange(
                            "p a b -> p (a b)"),
                        op=Al.min)
                    for k in range(4):
                        jt = qt * 4 + k
                        nc.tensor.matmul(ps_agg, h1_all[:, jt, h, :],
                                         e4[:, k, :],
                                         start=(n_mm == 0),
                                         stop=(n_mm == JT - 1))
                        n_mm += 1
                rz = work.tile([1, R], f16, tag="rz", bufs=2)
                with nc.allow_low_precision(reason="1/z in fp16: 1e-3 rel ok"):
                    nc.vector.reciprocal(rz, ps_agg[HID:HID + 1, :])
                ps_rzb = ps_bc.tile([128, R], f32, tag="bc")
                nc.tensor.matmul(ps_rzb[0:HID, :], ones16[:, 0:HID], rz,
                                 start=True, stop=True)
                rzb_sb = work.tile([HID, R], f16, tag="rzb", bufs=2)
                nc.scalar.copy(rzb_sb, ps_rzb[0:HID, :])
                y_h = work.tile([HID, R], f16, tag="yh", bufs=2)
                nc.vector.tensor_mul(y_h, ps_agg[0:HID, :], rzb_sb)
                po = (h % 2) * HID
                nc.scalar.activation(
                    x2T_all[po:po + HID, h // 2, :], y_h, A.Prelu,
                    bias=b1_sb[:, h:h + 1], alpha=NEG_OUT)

            # ---- layer 2: h2 per it, bounce, single AllGather ----
            bounce_in = dram.tile([R, G], f32, name="bin")
            bounce_out = dram.tile([N_CORES, R, G], f32,
                                   addr_space="Shared", name="bout")
            for it in range(IT):
                ps_h2t = ps_mm.tile([128, R], f32, tag="h")
                ps_h2 = ps_h2t[:, 0:OUT + 2]
                for kt in range(4):
                    nc.tensor.matmul(
                        ps_h2,
                        x2T_all[:, kt, it * 128:(it + 1) * 128],
                        rhs2_sb[:, kt, :],
                        start=(kt == 0), stop=False)
                nc.tensor.matmul(ps_h2, ones16, b2r_sb,
                                 start=False, stop=True)
                h2m = work.tile([128, G], f32, tag="h2m", bufs=2)
                nc.vector.tensor_copy(h2m[:, 0:HID].bitcast(f16),
                                      ps_h2[:, 0:OUT])
                nc.scalar.activation(h2m[:, HID:HID + 1],
                                     ps_h2[:, OUT:OUT + 1], A.Exp)
                nc.scalar.activation(h2m[:, HID + 1:HID + 2],
                                     ps_h2[:, OUT:OUT + 1], A.Exp,
                                     scale=NEG_ATT)
                nc.sync.dma_start(
                    out=bounce_in[it * 128:(it + 1) * 128, :], in_=h2m)
                ad2m = work.tile([128, 1], f32, tag="ad2m", bufs=2)
                nc.scalar.copy(ad2m, ps_h2[:, OUT + 1:OUT + 2])
                ps_adT2 = ps_sm.tile([1, 128], f32, tag="sm")
                nc.tensor.transpose(ps_adT2, ad2m, ident)
                nc.vector.tensor_copy(adst2T[:, it * 128:(it + 1) * 128],
                                      ps_adT2)
            nc.gpsimd.collective_compute(
                "AllGather",
                bass.mybir.AluOpType.bypass,
                replica_groups=[list(range(N_CORES))],
                ins=[bounce_in.opt()],
                outs=[bounce_out.opt()],
            )
            for c8 in range(N_CORES):
                nc.sync.dma_start(
                    out=h2g_all[:, c8, :, :],
                    in_=bounce_out[c8].rearrange("(r1 p) g -> p r1 g", p=128))

            # ---- layer-2 attention ----
            u8row2 = work.tile([1, R], f16, tag="u8row", bufs=2)
            nc.scalar.activation(u8row2, adst2T, A.Exp, scale=1.0 - NEG_ATT)
            ps_u82 = ps_bc.tile([128, R], f32, tag="bc")
            nc.tensor.matmul(ps_u82, ones16, u8row2, start=True, stop=True)
            nc.vector.tensor_copy(u8bc2, ps_u82)

            # reuse L1 pools: o2 in ps_mm "h" shape, z2 rides an "agg" buffer
            ps_o2 = ps_mm.tile([128, R], f32, tag="h")
            ps_z2t = ps_ag.tile([HID + 1, R], f32, tag="agg")
            ps_z2 = ps_z2t[HID:HID + 1, :]
            for qt in range(QT):
                q4 = qpool.tile([128, 4, R], f16, tag="q")
                for k in range(4):
                    jt = qt * 4 + k
                    c8, r1 = jt // IT, jt % IT
                    nc.vector.tensor_scalar(
                        q4[:, k, :], u8bc2,
                        h2g_all[:, c8, r1, HID:HID + 1],
                        h2g_all[:, c8, r1, HID + 1:HID + 2],
                        op0=Al.mult, op1=Al.max)
                e4 = epool.tile([128, 4, R], f16, tag="e")
                nc.vector.tensor_tensor(
                    e4.rearrange("p a b -> p (a b)"),
                    q4.rearrange("p a b -> p (a b)"),
                    adjT_all[:, qt * 4:(qt + 1) * 4, :].rearrange(
                        "p a b -> p (a b)"),
                    op=Al.min)
                for k in range(4):
                    jt = qt * 4 + k
                    c8, r1 = jt // IT, jt % IT
                    nc.tensor.matmul(
                        ps_o2, h2g_all[:, c8, r1, 0:HID].bitcast(f16),
                        e4[:, k, :],
                        start=(jt == 0), stop=(jt == JT - 1))
                    nc.tensor.matmul(
                        ps_z2, ones_col16, e4[:, k, :],
                        start=(jt == 0), stop=(jt == JT - 1))
            # per-it: transpose z2 chunk -> recip col; transpose o2 -> prelu
            o2sb = work.tile([128, R], f32, tag="o2sb", bufs=1)
            for it in range(IT):
                nc.scalar.copy(o2sb[:, it * 128:(it + 1) * 128],
                               ps_o2[:, it * 128:(it + 1) * 128])
            z2sb = work.tile([1, R], f16, tag="z2sb", bufs=1)
            nc.vector.tensor_copy(z2sb, ps_z2)
            del ps_z2t
            outT_sb = work.tile([128, IT, OUT], f32, tag="outT", bufs=1)
            for it in range(IT):
                isl = slice(it * 128, (it + 1) * 128)
                ps_zTt = ps_sm.tile([128, 128], f32, tag="sm")
                ps_zT = ps_zTt.bitcast(f16)[:, 0:1]
                nc.tensor.transpose(ps_zT, z2sb[:, isl], ident16[0:1, 0:1])
                with nc.allow_low_precision(reason="1/z2 col fp16 src ok"):
                    nc.vector.reciprocal(rz2col[:, it:it + 1], ps_zT)
                ps_oT = ps_sm.tile([128, 128], f32, tag="sm")
                nc.tensor.transpose(ps_oT, o2sb[:, isl], ident)
                nc.scalar.activation(outT_sb[:, it, :], ps_oT, A.Prelu,
                                     scale=rz2col[:, it:it + 1], alpha=NEG_OUT)
            nc.sync.dma_start(
                out=d_out.rearrange("(i p) c -> p i c", p=128), in_=outT_sb)

    nc.finalize()
    return nc


def _prep_host(x, adj, w1, att_src1, att_dst1, b1, w2, att_src2, att_dst2, b2):
    x = np.asarray(x, np.float32).reshape(N, F_IN)
    adj = np.asarray(adj, np.float32).reshape(N, N)
    w1 = np.asarray(w1, np.float32)
    w2 = np.asarray(w2, np.float32)
    att_src1 = np.asarray(att_src1, np.float32)
    att_dst1 = np.asarray(att_dst1, np.float32)
    att_src2 = np.asarray(att_src2, np.float32)
    att_dst2 = np.asarray(att_dst2, np.float32)
    b1 = np.asarray(b1, np.float32)
    b2 = np.asarray(b2, np.float32)

    xT = np.ascontiguousarray(x.T)
    xT16 = xT.astype(np.float16)
    adjm = (adj * MASKV).astype(np.float16)
    v_src1 = np.empty((F_IN, H1), np.float32)
    v_dst1 = np.empty((F_IN, H1), np.float32)
    for h in range(H1):
        blk = w1[:, h * HID:(h + 1) * HID]
        v_src1[:, h] = blk @ att_src1[h]
        v_dst1[:, h] = blk @ att_dst1[h]
    rhs1 = np.ascontiguousarray(
        np.concatenate([w1, v_src1], axis=1)).astype(np.float16)
    v_src2 = (w2 @ att_src2[0])[:, None]
    v_dst2 = (w2 @ att_dst2[0])[:, None]
    rhs2 = np.ascontiguousarray(
        np.concatenate([w2, v_src2, v_dst2], axis=1)).astype(np.float16)
    b1c = np.ascontiguousarray(b1.reshape(H1, HID).T)
    b2r = np.zeros((1, OUT + 2), np.float16)
    b2r[0, 0:OUT] = b2

    in_maps = []
    for c in range(N_CORES):
        rows = slice(c * R, (c + 1) * R)
        in_maps.append({
            "xT16": xT16,
            "xmT": np.ascontiguousarray(xT16[:, rows]),
            "adjT": np.ascontiguousarray(adjm[rows, :].T),
            "rhs1": rhs1,
            "vdst1": v_dst1.astype(np.float16),
            "rhs2": rhs2,
            "b1c": b1c,
            "b2r": b2r,
        })
    return in_maps


def kernel(**inputs) -> np.ndarray:
    from concourse.bass_utils import run_bass_kernel_spmd

    if "nc" not in _CACHE:
        _CACHE["nc"] = _build()
    nc = _CACHE["nc"]
    in_maps = _prep_host(**inputs)
    try:
        res = run_bass_kernel_spmd(nc, in_maps, list(range(N_CORES)))
    except Exception:
        # transient NRT device wedge — one clean retry
        res = run_bass_kernel_spmd(nc, in_maps, list(range(N_CORES)))
    out = np.empty((1, N, OUT), np.float32)
    for c in range(N_CORES):
        out[0, c * R:(c + 1) * R, :] = res.results[c]["outR"]
    return out


# revision 34
# speedup vs baseline: 1.0706x; 1.0065x over previous
"""Dense GAT (2-layer, 8+1 heads) on 8 Trainium2 NeuronCores — V3.

Row-parallel over destination rows i (R=512 per core). Per core:
  - adjacency arrives HOST-TRANSPOSED as adjT[j, i] in {0, 65504} fp16
    (mask applied via tensor MIN, no PE transposes needed).
  - h1|a_src from one fp16 matmul chain against host-folded
    [w1 | w1@blockdiag(att_src1)]; fp16 PSUM.
  - scaled-attention trick: softmax over j is invariant to any per-i
    factor, so E is normalized by exp(0.2*ad_i):
        e~[j,i] = min(adjT[j,i], max(eas_j * u8[i], e2as_j))
    with u8 = exp(0.8*ad_i) broadcast (one per head), eas = exp(as_j),
    e2as = exp(0.2*as_j) per-partition scalars.
    DVE chain: one tensor_scalar (4x mode) + mask-min (quad-batched).
    ACT chain (some tiles): Relu(u8*eas - e2as) + Identity(r + e2as).
    Mask-min on DVE or GPSIMD (Pool) per static schedule.
  - softmax denominators ride as a ones column in the aggregation lhsT.
  - one AllGather of [512, 66] f32 (h2+b2 packed fp16 | eas2 | e2as2).
  - L2 output normalized via per-partition ACT scale after PE transpose.
"""
import numpy as np

N = 4096
F_IN = 256
HID = 64
H1 = 8
F1 = H1 * HID
OUT = 128
N_CORES = 8
R = N // N_CORES
JT = N // 128          # 32 j-tiles
IT = R // 128          # 4 i-tiles
QT = JT // 4           # 8 quads of 4 j-tiles
NEG_ATT = 0.2
NEG_OUT = 0.01
MASKV = 65504.0        # fp16 max: adjacency "1" value; mask via min()

G = HID + 2            # bounce cols: 64 f32 words (128 f16 h2) | eas2 | e2as2

_CACHE = {}

# ---- static engine schedule knobs ----
# ACT chain quads: (h, qt) pairs routed to the scalar engine (2 ACT ops/tile)
ACT_QUADS = 14         # of 64 L1 quads on the ACT chain
_H1MOD = 2


def _is_act_quad(h, qt):
    return (h * QT + qt) * 7 % 64 < ACT_QUADS


def _build():
    import concourse.bass as bass
    from concourse import bacc
    import concourse.mybir as mybir
    import concourse.tile as tile
    from concourse.masks import make_identity

    f32 = mybir.dt.float32
    f16 = mybir.dt.float16
    A = mybir.ActivationFunctionType
    Al = mybir.AluOpType

    nc = bacc.Bacc("TRN2", target_bir_lowering=False, debug=False,
                   num_devices=N_CORES)
    d_xT16 = nc.dram_tensor("xT16", [F_IN, N], f16, kind="ExternalInput")
    d_xmT = nc.dram_tensor("xmT", [F_IN, R], f16, kind="ExternalInput")
    d_adjT = nc.dram_tensor("adjT", [N, R], f16, kind="ExternalInput")
    d_rhs1 = nc.dram_tensor("rhs1", [F_IN, F1 + H1], f16, kind="ExternalInput")
    d_vdst1 = nc.dram_tensor("vdst1", [F_IN, H1], f16, kind="ExternalInput")
    d_rhs2 = nc.dram_tensor("rhs2", [F1, OUT + 2], f16, kind="ExternalInput")
    d_b1c = nc.dram_tensor("b1c", [HID, H1], f32, kind="ExternalInput")
    d_b2r = nc.dram_tensor("b2r", [1, OUT + 2], f16, kind="ExternalInput")
    d_out = nc.dram_tensor("outR", [R, OUT], f32, kind="ExternalOutput")

    with tile.TileContext(nc) as tc:
        with tc.tile_pool(name="const", bufs=1) as const, \
             tc.tile_pool(name="big", bufs=1) as big, \
             tc.tile_pool(name="work", bufs=3) as work, \
             tc.tile_pool(name="qpool", bufs=4) as qpool, \
             tc.tile_pool(name="epool", bufs=4) as epool, \
             tc.tile_pool(name="dram", bufs=1, space="DRAM") as dram, \
             tc.tile_pool(name="ps_mm", bufs=2, space="PSUM") as ps_mm, \
             tc.tile_pool(name="ps_bc", bufs=2, space="PSUM") as ps_bc, \
             tc.tile_pool(name="ps_ag", bufs=2, space="PSUM") as ps_ag, \
             tc.tile_pool(name="ps_sm", bufs=2, space="PSUM") as ps_sm:
            ident = const.tile([128, 128], f32)
            make_identity(nc, ident)
            ident16 = const.tile([128, 128], f16)
            nc.vector.tensor_copy(ident16, ident)
            ones16 = const.tile([1, 128], f16)
            nc.vector.memset(ones16, 1.0)
            ones_col16 = const.tile([128, 1], f16)
            nc.vector.memset(ones_col16, 1.0)
            rhs1_sb = const.tile([128, 2, F1 + H1], f16)
            nc.sync.dma_start(out=rhs1_sb[:, 0, :], in_=d_rhs1[0:128, :])
            nc.sync.dma_start(out=rhs1_sb[:, 1, :], in_=d_rhs1[128:256, :])
            vdst1_sb = const.tile([128, 2, H1], f16)
            nc.sync.dma_start(out=vdst1_sb[:, 0, :], in_=d_vdst1[0:128, :])
            nc.sync.dma_start(out=vdst1_sb[:, 1, :], in_=d_vdst1[128:256, :])
            rhs2_sb = const.tile([128, 4, OUT + 2], f16)
            for kt in range(4):
                nc.sync.dma_start(out=rhs2_sb[:, kt, :],
                                  in_=d_rhs2[kt * 128:(kt + 1) * 128, :])
            b1_sb = const.tile([HID, H1], f32)
            nc.sync.dma_start(out=b1_sb, in_=d_b1c[:, :])
            b2r_sb = const.tile([1, OUT + 2], f16)
            nc.sync.dma_start(out=b2r_sb, in_=d_b2r[:, :])

            # ---- big persistent arrays ----
            adjT_all = big.tile([128, JT, R], f16)       # 32 KB/part
            xT_sb = big.tile([128, 2, N], f16)           # 16 KB/part
            h1_all = big.tile([128, JT, H1, HID + 1], f16)  # 32.5 KB/part
            asrc16 = big.tile([128, JT, H1], f16)
            easrc = big.tile([128, JT, H1], f32)
            e2src = big.tile([128, JT, H1], f32)
            nege2 = big.tile([128, JT, H1], f32)
            adstT = big.tile([H1, R], f32)
            adst_rows = big.tile([1, H1, R], f32)
            adst2T = big.tile([1, R], f32)
            x2T_all = big.tile([128, 4, R], f16)
            u8bc = big.tile([128, 2, R], f16)            # 2-head pipeline
            h2g_all = big.tile([128, N_CORES, IT, G], f32)
            u8bc2 = big.tile([128, R], f16)
            rz2col = big.tile([128, IT], f32)

            nc.vector.memset(h1_all[:, :, :, HID:HID + 1], 1.0)

            # ---- input DMAs: xmT/xT first (a_dst + h1 need them), adjT after
            xmT_sb = big.tile([128, 2, R], f16)
            nc.sync.dma_start(out=xmT_sb[:, 0, :], in_=d_xmT[0:128, :])
            nc.sync.dma_start(out=xmT_sb[:, 1, :], in_=d_xmT[128:256, :])
            for c4 in range(4):
                csl = slice(c4 * (N // 4), (c4 + 1) * (N // 4))
                nc.sync.dma_start(out=xT_sb[:, 0, csl], in_=d_xT16[0:128, csl])
                nc.sync.dma_start(out=xT_sb[:, 1, csl],
                                  in_=d_xT16[128:256, csl])
            for jb in range(8):
                nc.sync.dma_start(
                    out=adjT_all[:, jb, :],
                    in_=d_adjT[jb * 128:(jb + 1) * 128, :])

            # ---- a_dst (own rows) -> adstT [H1, R] f32 ----
            for it in range(IT):
                ps_ad = ps_sm.tile([128, 128], f32, tag="sm")
                for kb in range(2):
                    nc.tensor.matmul(ps_ad[:, 0:H1],
                                     xmT_sb[:, kb, it * 128:(it + 1) * 128],
                                     vdst1_sb[:, kb, :],
                                     start=(kb == 0), stop=(kb == 1))
                adm = work.tile([128, H1], f32, tag="adm", bufs=2)
                nc.vector.tensor_copy(adm, ps_ad[:, 0:H1])
                ps_adT = ps_sm.tile([128, 128], f32, tag="sm")
                nc.tensor.transpose(ps_adT[0:H1, :], adm, ident)
                nc.vector.tensor_copy(adstT[:, it * 128:(it + 1) * 128],
                                      ps_adT[0:H1, :])
            for h in range(H1):
                nc.sync.dma_start(out=adst_rows[:, h, :], in_=adstT[h:h + 1, :])
            for jb in range(8, JT):
                nc.sync.dma_start(
                    out=adjT_all[:, jb, :],
                    in_=d_adjT[jb * 128:(jb + 1) * 128, :])

            # ---- h1 | a_src per jt ----
            for jt in range(JT):
                cols = slice(jt * 128, (jt + 1) * 128)
                ps_h = ps_mm.tile([128, F1], f32, tag="h")
                ps_ast = ps_sm.tile([128, 128], f32, tag="sm")
                ps_as = ps_ast[:, 0:H1]
                for kb in range(2):
                    nc.tensor.matmul(ps_h, xT_sb[:, kb, cols],
                                     rhs1_sb[:, kb, 0:F1],
                                     start=(kb == 0), stop=(kb == 1))
                    nc.tensor.matmul(ps_as, xT_sb[:, kb, cols],
                                     rhs1_sb[:, kb, F1:F1 + H1],
                                     start=(kb == 0), stop=(kb == 1))
                # h1 [j, h, c] copies: 1 in _H1MOD on DVE, rest on ACT
                if jt % _H1MOD == 0:
                    nc.vector.tensor_copy(
                        h1_all[:, jt, :, 0:HID],
                        ps_h.rearrange("p (h c) -> p h c", c=HID))
                else:
                    nc.scalar.copy(
                        h1_all[:, jt, :, 0:HID],
                        ps_h.rearrange("p (h c) -> p h c", c=HID))
                nc.scalar.copy(asrc16[:, jt, :], ps_as)
                if jt % 8 == 7:
                    gs = slice(jt - 7, jt + 1)
                    nc.scalar.activation(easrc[:, gs, :], asrc16[:, gs, :],
                                         A.Exp)
                    nc.scalar.activation(e2src[:, gs, :], asrc16[:, gs, :],
                                         A.Exp, scale=NEG_ATT)
                    nc.scalar.activation(nege2[:, gs, :], e2src[:, gs, :],
                                         A.Identity, scale=-1.0)

            # ---- layer-1 attention, head-pipelined ----
            def _pre_head(h):
                u8row = work.tile([1, R], f16, tag="u8row", bufs=2,
                                  name=f"u8r{h}")
                nc.scalar.activation(u8row, adst_rows[:, h, :], A.Exp,
                                     scale=1.0 - NEG_ATT)
                ps_u8 = ps_bc.tile([128, R], f32, tag="bc", name=f"psu8{h}")
                nc.tensor.matmul(ps_u8, ones16, u8row, start=True, stop=True)
                nc.vector.tensor_copy(u8bc[:, h % 2, :], ps_u8)

            _pre_head(0)
            for h in range(H1):
                if h + 1 < H1:
                    _pre_head(h + 1)
                u8 = u8bc[:, h % 2, :]
                ps_agg = ps_ag.tile([HID + 1, R], f32, tag="agg")
                act_qts = [qt for qt in range(QT) if _is_act_quad(h, qt)]
                dve_qts = [qt for qt in range(QT) if not _is_act_quad(h, qt)]
                # ACT-chain q tiles first so the scalar engine runs ahead
                act_q4 = {}
                for qt in act_qts:
                    q4 = qpool.tile([128, 4, R], f16, tag="qa", bufs=3,
                                    name=f"qa{h}_{qt}")
                    for k in range(4):
                        jt = qt * 4 + k
                        r = work.tile([128, R], f16, tag="ract", bufs=3)
                        nc.scalar.activation(
                            r, u8, A.Relu,
                            bias=nege2[:, jt, h:h + 1],
                            scale=easrc[:, jt, h:h + 1])
                        nc.scalar.activation(
                            q4[:, k, :], r, A.Identity,
                            bias=e2src[:, jt, h:h + 1])
                    act_q4[qt] = q4
                n_mm = 0
                for qt in dve_qts + act_qts:
                    if qt in act_q4:
                        q4 = act_q4[qt]
                    else:
                        q4 = qpool.tile([128, 4, R], f16, tag="q")
                        for k in range(4):
                            jt = qt * 4 + k
                            nc.vector.tensor_scalar(
                                q4[:, k, :], u8,
                                easrc[:, jt, h:h + 1],
                                e2src[:, jt, h:h + 1],
                                op0=Al.mult, op1=Al.max)
                    e4 = epool.tile([128, 4, R], f16, tag="e")
                    nc.vector.tensor_tensor(
                        e4.rearrange("p a b -> p (a b)"),
                        q4.rearrange("p a b -> p (a b)"),
                        adjT_all[:, qt * 4:(qt + 1) * 4, :].rearrange(
                            "p a b -> p (a b)"),
                        op=Al.min)
                    for k in range(4):
                        jt = qt * 4 + k
                        nc.tensor.matmul(ps_agg, h1_all[:, jt, h, :],
                                         e4[:, k, :],
                                         start=(n_mm == 0),
                                         stop=(n_mm == JT - 1))
                        n_mm += 1
                rz = work.tile([1, R], f16, tag="rz", bufs=2)
                with nc.allow_low_precision(reason="1/z in fp16: 1e-3 rel ok"):
                    nc.vector.reciprocal(rz, ps_agg[HID:HID + 1, :])
                ps_rzb = ps_bc.tile([128, R], f32, tag="bc")
                nc.tensor.matmul(ps_rzb[0:HID, :], ones16[:, 0:HID], rz,
                                 start=True, stop=True)
                rzb_sb = work.tile([HID, R], f16, tag="rzb", bufs=2)
                nc.scalar.copy(rzb_sb, ps_rzb[0:HID, :])
                y_h = work.tile([HID, R], f16, tag="yh", bufs=2)
                nc.vector.tensor_mul(y_h, ps_agg[0:HID, :], rzb_sb)
                po = (h % 2) * HID
                nc.scalar.activation(
                    x2T_all[po:po + HID, h // 2, :], y_h, A.Prelu,
                    bias=b1_sb[:, h:h + 1], alpha=NEG_OUT)

            # ---- layer 2: h2 per it, bounce, single AllGather ----
            bounce_in = dram.tile([R, G], f32, name="bin")
            bounce_out = dram.tile([N_CORES, R, G], f32,
                                   addr_space="Shared", name="bout")
            for it in range(IT):
                ps_h2t = ps_mm.tile([128, R], f32, tag="h")
                ps_h2 = ps_h2t[:, 0:OUT + 2]
                for kt in range(4):
                    nc.tensor.matmul(
                        ps_h2,
                        x2T_all[:, kt, it * 128:(it + 1) * 128],
                        rhs2_sb[:, kt, :],
                        start=(kt == 0), stop=False)
                nc.tensor.matmul(ps_h2, ones16, b2r_sb,
                                 start=False, stop=True)
                h2m = work.tile([128, G], f32, tag="h2m", bufs=2)
                nc.vector.tensor_copy(h2m[:, 0:HID].bitcast(f16),
                                      ps_h2[:, 0:OUT])
                nc.scalar.activation(h2m[:, HID:HID + 1],
                                     ps_h2[:, OUT:OUT + 1], A.Exp)
                nc.scalar.activation(h2m[:, HID + 1:HID + 2],
                                     ps_h2[:, OUT:OUT + 1], A.Exp,
                                     scale=NEG_ATT)
                nc.sync.dma_start(
                    out=bounce_in[it * 128:(it + 1) * 128, :], in_=h2m)
                ad2m = work.tile([128, 1], f32, tag="ad2m", bufs=2)
                nc.scalar.copy(ad2m, ps_h2[:, OUT + 1:OUT + 2])
                ps_adT2 = ps_sm.tile([1, 128], f32, tag="sm")
                nc.tensor.transpose(ps_adT2, ad2m, ident)
                nc.vector.tensor_copy(adst2T[:, it * 128:(it + 1) * 128],
                                      ps_adT2)
            nc.gpsimd.collective_compute(
                "AllGather",
                bass.mybir.AluOpType.bypass,
                replica_groups=[list(range(N_CORES))],
                ins=[bounce_in.opt()],
                outs=[bounce_out.opt()],
            )
            for c8 in range(N_CORES):
                nc.sync.dma_start(
                    out=h2g_all[:, c8, :, :],
                    in_=bounce_out[c8].rearrange("(r1 p) g -> p r1 g", p=128))

            # ---- layer-2 attention ----
            u8row2 = work.tile([1, R], f16, tag="u8row", bufs=2)
            nc.scalar.activation(u8row2, adst2T, A.Exp, scale=1.0 - NEG_ATT)
            ps_u82 = ps_bc.tile([128, R], f32, tag="bc")
            nc.tensor.matmul(ps_u82, ones16, u8row2, start=True, stop=True)
            nc.vector.tensor_copy(u8bc2, ps_u82)

            # reuse L1 pools: o2 in ps_mm "h" shape, z2 rides an "agg" buffer
            ps_o2 = ps_mm.tile([128, R], f32, tag="h")
            ps_z2t = ps_ag.tile([HID + 1, R], f32, tag="agg")
            ps_z2 = ps_z2t[HID:HID + 1, :]
            for qt in range(QT):
                q4 = qpool.tile([128, 4, R], f16, tag="q")
                for k in range(4):
                    jt = qt * 4 + k
                    c8, r1 = jt // IT, jt % IT
                    nc.vector.tensor_scalar(
                        q4[:, k, :], u8bc2,
                        h2g_all[:, c8, r1, HID:HID + 1],
                        h2g_all[:, c8, r1, HID + 1:HID + 2],
                        op0=Al.mult, op1=Al.max)
                e4 = epool.tile([128, 4, R], f16, tag="e")
                nc.vector.tensor_tensor(
                    e4.rearrange("p a b -> p (a b)"),
                    q4.rearrange("p a b -> p (a b)"),
                    adjT_all[:, qt * 4:(qt + 1) * 4, :].rearrange(
                        "p a b -> p (a b)"),
                    op=Al.min)
                for k in range(4):
                    jt = qt * 4 + k
                    c8, r1 = jt // IT, jt % IT
                    nc.tensor.matmul(
                        ps_o2, h2g_all[:, c8, r1, 0:HID].bitcast(f16),
                        e4[:, k, :],
                        start=(jt == 0), stop=(jt == JT - 1))
                    nc.tensor.matmul(
                        ps_z2, ones_col16, e4[:, k, :],
                        start=(jt == 0), stop=(jt == JT - 1))
            # per-it: transpose z2 chunk -> recip col; transpose o2 -> prelu
            o2sb = work.tile([128, R], f32, tag="o2sb", bufs=1)
            for it in range(IT):
                nc.scalar.copy(o2sb[:, it * 128:(it + 1) * 128],
                               ps_o2[:, it * 128:(it + 1) * 128])
            z2sb = work.tile([1, R], f16, tag="z2sb", bufs=1)
            nc.vector.tensor_copy(z2sb, ps_z2)
            del ps_z2t
            outT_sb = work.tile([128, IT, OUT], f32, tag="outT", bufs=1)
            for it in range(IT):
                isl = slice(it * 128, (it + 1) * 128)
                ps_zTt = ps_sm.tile([128, 128], f32, tag="sm")
                ps_zT = ps_zTt.bitcast(f16)[:, 0:1]
                nc.tensor.transpose(ps_zT, z2sb[:, isl], ident16[0:1, 0:1])
                with nc.allow_low_precision(reason="1/z2 col fp16 src ok"):
                    nc.vector.reciprocal(rz2col[:, it:it + 1], ps_zT)
                ps_oT = ps_sm.tile([128, 128], f32, tag="sm")
                nc.tensor.transpose(ps_oT, o2sb[:, isl], ident)
                nc.scalar.activation(outT_sb[:, it, :], ps_oT, A.Prelu,
                                     scale=rz2col[:, it:it + 1], alpha=NEG_OUT)
            nc.sync.dma_start(
                out=d_out.rearrange("(i p) c -> p i c", p=128), in_=outT_sb)

    nc.finalize()
    return nc


def _prep_host(x, adj, w1, att_src1, att_dst1, b1, w2, att_src2, att_dst2, b2):
    x = np.asarray(x, np.float32).reshape(N, F_IN)
    adj = np.asarray(adj, np.float32).reshape(N, N)
    w1 = np.asarray(w1, np.float32)
    w2 = np.asarray(w2, np.float32)
    att_src1 = np.asarray(att_src1, np.float32)
    att_dst1 = np.asarray(att_dst1, np.float32)
    att_src2 = np.asarray(att_src2, np.float32)
    att_dst2 = np.asarray(att_dst2, np.float32)
    b1 = np.asarray(b1, np.float32)
    b2 = np.asarray(b2, np.float32)

    xT = np.ascontiguousarray(x.T)
    xT16 = xT.astype(np.float16)
    adjm = (adj * MASKV).astype(np.float16)
    v_src1 = np.empty((F_IN, H1), np.float32)
    v_dst1 = np.empty((F_IN, H1), np.float32)
    for h in range(H1):
        blk = w1[:, h * HID:(h + 1) * HID]
        v_src1[:, h] = blk @ att_src1[h]
        v_dst1[:, h] = blk @ att_dst1[h]
    rhs1 = np.ascontiguousarray(
        np.concatenate([w1, v_src1], axis=1)).astype(np.float16)
    v_src2 = (w2 @ att_src2[0])[:, None]
    v_dst2 = (w2 @ att_dst2[0])[:, None]
    rhs2 = np.ascontiguousarray(
        np.concatenate([w2, v_src2, v_dst2], axis=1)).astype(np.float16)
    b1c = np.ascontiguousarray(b1.reshape(H1, HID).T)
    b2r = np.zeros((1, OUT + 2), np.float16)
    b2r[0, 0:OUT] = b2

    in_maps = []
    for c in range(N_CORES):
        rows = slice(c * R, (c + 1) * R)
        in_maps.append({
            "xT16": xT16,
            "xmT": np.ascontiguousarray(xT16[:, rows]),
            "adjT": np.ascontiguousarray(adjm[rows, :].T),
            "rhs1": rhs1,
            "vdst1": v_dst1.astype(np.float16),
            "rhs2": rhs2,
            "b1c": b1c,
            "b2r": b2r,
        })
    return in_maps


def kernel(**inputs) -> np.ndarray:
    from concourse.bass_utils import run_bass_kernel_spmd

    if "nc" not in _CACHE:
        _CACHE["nc"] = _build()
    nc = _CACHE["nc"]
    in_maps = _prep_host(**inputs)
    try:
        res = run_bass_kernel_spmd(nc, in_maps, list(range(N_CORES)))
    except Exception:
        # transient NRT device wedge — one clean retry
        res = run_bass_kernel_spmd(nc, in_maps, list(range(N_CORES)))
    out = np.empty((1, N, OUT), np.float32)
    for c in range(N_CORES):
        out[0, c * R:(c + 1) * R, :] = res.results[c]["outR"]
    return out


# revision 37
# speedup vs baseline: 1.1029x; 1.0302x over previous
"""Dense GAT (2-layer, 8+1 heads) on 8 Trainium2 NeuronCores — V3.

Row-parallel over destination rows i (R=512 per core). Per core:
  - adjacency arrives HOST-TRANSPOSED as adjT[j, i] in {0, 65504} fp16
    (mask applied via tensor MIN, no PE transposes needed).
  - h1|a_src from one fp16 matmul chain against host-folded
    [w1 | w1@blockdiag(att_src1)]; fp16 PSUM.
  - scaled-attention trick: softmax over j is invariant to any per-i
    factor, so E is normalized by exp(0.2*ad_i):
        e~[j,i] = min(adjT[j,i], max(eas_j * u8[i], e2as_j))
    with u8 = exp(0.8*ad_i) broadcast (one per head), eas = exp(as_j),
    e2as = exp(0.2*as_j) per-partition scalars.
    DVE chain: one tensor_scalar (4x mode) + mask-min (quad-batched).
    ACT chain (some tiles): Relu(u8*eas - e2as) + Identity(r + e2as).
    Mask-min on DVE or GPSIMD (Pool) per static schedule.
  - softmax denominators ride as a ones column in the aggregation lhsT.
  - one AllGather of [512, 66] f32 (h2+b2 packed fp16 | eas2 | e2as2).
  - L2 output normalized via per-partition ACT scale after PE transpose.
"""
import numpy as np

N = 4096
F_IN = 256
HID = 64
H1 = 8
F1 = H1 * HID
OUT = 128
N_CORES = 8
R = N // N_CORES
JT = N // 128          # 32 j-tiles
IT = R // 128          # 4 i-tiles
QT = JT // 4           # 8 quads of 4 j-tiles
NEG_ATT = 0.2
NEG_OUT = 0.01
MASKV = 65504.0        # fp16 max: adjacency "1" value; mask via min()

G = HID + 2            # bounce cols: 64 f32 words (128 f16 h2) | eas2 | e2as2

_CACHE = {}

# ---- static engine schedule knobs ----
# ACT chain quads: (h, qt) pairs routed to the scalar engine (2 ACT ops/tile)
# per-head ACT-chain quad quota (ACT has h1-copy work early, idles late)
ACT_QUOTA = [1, 1, 2, 2, 2, 2, 3, 3]
_H1MOD = 2


def _is_act_quad(h, qt):
    return (qt * 5 + h) % 8 < ACT_QUOTA[h]


def _build():
    import concourse.bass as bass
    from concourse import bacc
    import concourse.mybir as mybir
    import concourse.tile as tile
    from concourse.masks import make_identity

    f32 = mybir.dt.float32
    f16 = mybir.dt.float16
    A = mybir.ActivationFunctionType
    Al = mybir.AluOpType

    nc = bacc.Bacc("TRN2", target_bir_lowering=False, debug=False,
                   num_devices=N_CORES)
    d_xT16 = nc.dram_tensor("xT16", [F_IN, N], f16, kind="ExternalInput")
    d_xmT = nc.dram_tensor("xmT", [F_IN, R], f16, kind="ExternalInput")
    d_adjT = nc.dram_tensor("adjT", [N, R], f16, kind="ExternalInput")
    d_rhs1 = nc.dram_tensor("rhs1", [F_IN, F1 + H1], f16, kind="ExternalInput")
    d_vdst1 = nc.dram_tensor("vdst1", [F_IN, H1], f16, kind="ExternalInput")
    d_rhs2 = nc.dram_tensor("rhs2", [F1, OUT + 2], f16, kind="ExternalInput")
    d_b1c = nc.dram_tensor("b1c", [HID, H1], f32, kind="ExternalInput")
    d_b2r = nc.dram_tensor("b2r", [1, OUT + 2], f16, kind="ExternalInput")
    d_out = nc.dram_tensor("outR", [R, OUT], f32, kind="ExternalOutput")

    with tile.TileContext(nc) as tc:
        with tc.tile_pool(name="const", bufs=1) as const, \
             tc.tile_pool(name="big", bufs=1) as big, \
             tc.tile_pool(name="work", bufs=3) as work, \
             tc.tile_pool(name="qpool", bufs=4) as qpool, \
             tc.tile_pool(name="epool", bufs=4) as epool, \
             tc.tile_pool(name="dram", bufs=1, space="DRAM") as dram, \
             tc.tile_pool(name="ps_mm", bufs=2, space="PSUM") as ps_mm, \
             tc.tile_pool(name="ps_bc", bufs=2, space="PSUM") as ps_bc, \
             tc.tile_pool(name="ps_ag", bufs=2, space="PSUM") as ps_ag, \
             tc.tile_pool(name="ps_sm", bufs=2, space="PSUM") as ps_sm:
            ident = const.tile([128, 128], f32)
            make_identity(nc, ident)
            ident16 = const.tile([128, 128], f16)
            nc.vector.tensor_copy(ident16, ident)
            ones16 = const.tile([1, 128], f16)
            nc.vector.memset(ones16, 1.0)
            ones_col16 = const.tile([128, 1], f16)
            nc.vector.memset(ones_col16, 1.0)
            rhs1_sb = const.tile([128, 2, F1 + H1], f16)
            nc.sync.dma_start(out=rhs1_sb[:, 0, :], in_=d_rhs1[0:128, :])
            nc.sync.dma_start(out=rhs1_sb[:, 1, :], in_=d_rhs1[128:256, :])
            vdst1_sb = const.tile([128, 2, H1], f16)
            nc.sync.dma_start(out=vdst1_sb[:, 0, :], in_=d_vdst1[0:128, :])
            nc.sync.dma_start(out=vdst1_sb[:, 1, :], in_=d_vdst1[128:256, :])
            rhs2_sb = const.tile([128, 4, OUT + 2], f16)
            for kt in range(4):
                nc.sync.dma_start(out=rhs2_sb[:, kt, :],
                                  in_=d_rhs2[kt * 128:(kt + 1) * 128, :])
            b1_sb = const.tile([HID, H1], f32)
            nc.sync.dma_start(out=b1_sb, in_=d_b1c[:, :])
            b2r_sb = const.tile([1, OUT + 2], f16)
            nc.sync.dma_start(out=b2r_sb, in_=d_b2r[:, :])

            # ---- big persistent arrays ----
            adjT_all = big.tile([128, JT, R], f16)       # 32 KB/part
            xT_sb = big.tile([128, 2, N], f16)           # 16 KB/part
            h1_all = big.tile([128, JT, H1, HID + 1], f16)  # 32.5 KB/part
            asrc16 = big.tile([128, JT, H1], f16)
            easrc = big.tile([128, JT, H1], f32)
            e2src = big.tile([128, JT, H1], f32)
            nege2 = big.tile([128, JT, H1], f32)
            adstT = big.tile([H1, R], f32)
            adst_rows = big.tile([1, H1, R], f32)
            adst2T = big.tile([1, R], f32)
            x2T_all = big.tile([128, 4, R], f16)
            u8bc = big.tile([128, 2, R], f16)            # 2-head pipeline
            h2g_all = big.tile([128, N_CORES, IT, G], f32)
            u8bc2 = big.tile([128, R], f16)
            rz2col = big.tile([128, IT], f32)

            nc.vector.memset(h1_all[:, :, :, HID:HID + 1], 1.0)

            # ---- input DMAs: xmT/xT first (a_dst + h1 need them), adjT after
            xmT_sb = big.tile([128, 2, R], f16)
            nc.sync.dma_start(out=xmT_sb[:, 0, :], in_=d_xmT[0:128, :])
            nc.sync.dma_start(out=xmT_sb[:, 1, :], in_=d_xmT[128:256, :])
            for c4 in range(4):
                csl = slice(c4 * (N // 4), (c4 + 1) * (N // 4))
                nc.sync.dma_start(out=xT_sb[:, 0, csl], in_=d_xT16[0:128, csl])
                nc.sync.dma_start(out=xT_sb[:, 1, csl],
                                  in_=d_xT16[128:256, csl])
            for jb in range(8):
                nc.sync.dma_start(
                    out=adjT_all[:, jb, :],
                    in_=d_adjT[jb * 128:(jb + 1) * 128, :])

            # ---- a_dst (own rows) -> adstT [H1, R] f32 ----
            for it in range(IT):
                ps_ad = ps_sm.tile([128, 128], f32, tag="sm")
                for kb in range(2):
                    nc.tensor.matmul(ps_ad[:, 0:H1],
                                     xmT_sb[:, kb, it * 128:(it + 1) * 128],
                                     vdst1_sb[:, kb, :],
                                     start=(kb == 0), stop=(kb == 1))
                adm = work.tile([128, H1], f32, tag="adm", bufs=2)
                nc.vector.tensor_copy(adm, ps_ad[:, 0:H1])
                ps_adT = ps_sm.tile([128, 128], f32, tag="sm")
                nc.tensor.transpose(ps_adT[0:H1, :], adm, ident)
                nc.vector.tensor_copy(adstT[:, it * 128:(it + 1) * 128],
                                      ps_adT[0:H1, :])
            for h in range(H1):
                nc.sync.dma_start(out=adst_rows[:, h, :], in_=adstT[h:h + 1, :])
            for jb in range(8, JT):
                nc.sync.dma_start(
                    out=adjT_all[:, jb, :],
                    in_=d_adjT[jb * 128:(jb + 1) * 128, :])

            # ---- h1 | a_src per jt ----
            for jt in range(JT):
                cols = slice(jt * 128, (jt + 1) * 128)
                ps_h = ps_mm.tile([128, F1], f32, tag="h")
                ps_ast = ps_sm.tile([128, 128], f32, tag="sm")
                ps_as = ps_ast[:, 0:H1]
                for kb in range(2):
                    nc.tensor.matmul(ps_h, xT_sb[:, kb, cols],
                                     rhs1_sb[:, kb, 0:F1],
                                     start=(kb == 0), stop=(kb == 1))
                    nc.tensor.matmul(ps_as, xT_sb[:, kb, cols],
                                     rhs1_sb[:, kb, F1:F1 + H1],
                                     start=(kb == 0), stop=(kb == 1))
                # h1 [j, h, c] copies: 1 in _H1MOD on DVE, rest on ACT
                if jt % _H1MOD == 0:
                    nc.vector.tensor_copy(
                        h1_all[:, jt, :, 0:HID],
                        ps_h.rearrange("p (h c) -> p h c", c=HID))
                else:
                    nc.scalar.copy(
                        h1_all[:, jt, :, 0:HID],
                        ps_h.rearrange("p (h c) -> p h c", c=HID))
                nc.scalar.copy(asrc16[:, jt, :], ps_as)
                if jt % 8 == 7:
                    gs = slice(jt - 7, jt + 1)
                    nc.scalar.activation(easrc[:, gs, :], asrc16[:, gs, :],
                                         A.Exp)
                    nc.scalar.activation(e2src[:, gs, :], asrc16[:, gs, :],
                                         A.Exp, scale=NEG_ATT)
                    nc.scalar.activation(nege2[:, gs, :], e2src[:, gs, :],
                                         A.Identity, scale=-1.0)

            # ---- layer-1 attention, head-pipelined ----
            def _pre_head(h):
                u8row = work.tile([1, R], f16, tag="u8row", bufs=2,
                                  name=f"u8r{h}")
                nc.scalar.activation(u8row, adst_rows[:, h, :], A.Exp,
                                     scale=1.0 - NEG_ATT)
                ps_u8 = ps_bc.tile([128, R], f32, tag="bc", name=f"psu8{h}")
                nc.tensor.matmul(ps_u8, ones16, u8row, start=True, stop=True)
                nc.scalar.copy(u8bc[:, h % 2, :], ps_u8)

            _pre_head(0)
            for h in range(H1):
                if h + 1 < H1:
                    _pre_head(h + 1)
                u8 = u8bc[:, h % 2, :]
                ps_agg = ps_ag.tile([HID + 1, R], f32, tag="agg")
                act_qts = [qt for qt in range(QT) if _is_act_quad(h, qt)]
                dve_qts = [qt for qt in range(QT) if not _is_act_quad(h, qt)]
                # ACT-chain q tiles first so the scalar engine runs ahead
                act_q4 = {}
                for qt in act_qts:
                    q4 = qpool.tile([128, 4, R], f16, tag="qa", bufs=3,
                                    name=f"qa{h}_{qt}")
                    for k in range(4):
                        jt = qt * 4 + k
                        r = work.tile([128, R], f16, tag="ract", bufs=3)
                        nc.scalar.activation(
                            r, u8, A.Relu,
                            bias=nege2[:, jt, h:h + 1],
                            scale=easrc[:, jt, h:h + 1])
                        nc.scalar.activation(
                            q4[:, k, :], r, A.Identity,
                            bias=e2src[:, jt, h:h + 1])
                    act_q4[qt] = q4
                n_mm = 0
                for qt in dve_qts + act_qts:
                    if qt in act_q4:
                        q4 = act_q4[qt]
                    else:
                        q4 = qpool.tile([128, 4, R], f16, tag="q")
                        for k in range(4):
                            jt = qt * 4 + k
                            nc.vector.tensor_scalar(
                                q4[:, k, :], u8,
                                easrc[:, jt, h:h + 1],
                                e2src[:, jt, h:h + 1],
                                op0=Al.mult, op1=Al.max)
                    e4 = epool.tile([128, 4, R], f16, tag="e")
                    nc.vector.tensor_tensor(
                        e4.rearrange("p a b -> p (a b)"),
                        q4.rearrange("p a b -> p (a b)"),
                        adjT_all[:, qt * 4:(qt + 1) * 4, :].rearrange(
                            "p a b -> p (a b)"),
                        op=Al.min)
                    for k in range(4):
                        jt = qt * 4 + k
                        nc.tensor.matmul(ps_agg, h1_all[:, jt, h, :],
                                         e4[:, k, :],
                                         start=(n_mm == 0),
                                         stop=(n_mm == JT - 1))
                        n_mm += 1
                rz = work.tile([1, R], f16, tag="rz", bufs=2)
                with nc.allow_low_precision(reason="1/z in fp16: 1e-3 rel ok"):
                    nc.vector.reciprocal(rz, ps_agg[HID:HID + 1, :])
                ps_rzb = ps_bc.tile([128, R], f32, tag="bc")
                nc.tensor.matmul(ps_rzb[0:HID, :], ones16[:, 0:HID], rz,
                                 start=True, stop=True)
                rzb_sb = work.tile([HID, R], f16, tag="rzb", bufs=2)
                nc.scalar.copy(rzb_sb, ps_rzb[0:HID, :])
                y_h = work.tile([HID, R], f16, tag="yh", bufs=2)
                nc.vector.tensor_mul(y_h, ps_agg[0:HID, :], rzb_sb)
                po = (h % 2) * HID
                nc.scalar.activation(
                    x2T_all[po:po + HID, h // 2, :], y_h, A.Prelu,
                    bias=b1_sb[:, h:h + 1], alpha=NEG_OUT)

            # ---- layer 2: h2 per it, bounce, single AllGather ----
            bounce_in = dram.tile([R, G], f32, name="bin")
            bounce_out = dram.tile([N_CORES, R, G], f32,
                                   addr_space="Shared", name="bout")
            for it in range(IT):
                ps_h2t = ps_mm.tile([128, R], f32, tag="h")
                ps_h2 = ps_h2t[:, 0:OUT + 2]
                for kt in range(4):
                    nc.tensor.matmul(
                        ps_h2,
                        x2T_all[:, kt, it * 128:(it + 1) * 128],
                        rhs2_sb[:, kt, :],
                        start=(kt == 0), stop=False)
                nc.tensor.matmul(ps_h2, ones16, b2r_sb,
                                 start=False, stop=True)
                h2m = work.tile([128, G], f32, tag="h2m", bufs=2)
                nc.scalar.copy(h2m[:, 0:HID].bitcast(f16),
                               ps_h2[:, 0:OUT])
                nc.scalar.activation(h2m[:, HID:HID + 1],
                                     ps_h2[:, OUT:OUT + 1], A.Exp)
                nc.scalar.activation(h2m[:, HID + 1:HID + 2],
                                     ps_h2[:, OUT:OUT + 1], A.Exp,
                                     scale=NEG_ATT)
                nc.sync.dma_start(
                    out=bounce_in[it * 128:(it + 1) * 128, :], in_=h2m)
                ad2m = work.tile([128, 1], f32, tag="ad2m", bufs=2)
                nc.scalar.copy(ad2m, ps_h2[:, OUT + 1:OUT + 2])
                ps_adT2 = ps_sm.tile([1, 128], f32, tag="sm")
                nc.tensor.transpose(ps_adT2, ad2m, ident)
                nc.scalar.copy(adst2T[:, it * 128:(it + 1) * 128],
                               ps_adT2)
            nc.gpsimd.collective_compute(
                "AllGather",
                bass.mybir.AluOpType.bypass,
                replica_groups=[list(range(N_CORES))],
                ins=[bounce_in.opt()],
                outs=[bounce_out.opt()],
            )
            for c8 in range(N_CORES):
                nc.sync.dma_start(
                    out=h2g_all[:, c8, :, :],
                    in_=bounce_out[c8].rearrange("(r1 p) g -> p r1 g", p=128))

            # ---- layer-2 attention ----
            u8row2 = work.tile([1, R], f16, tag="u8row", bufs=2)
            nc.scalar.activation(u8row2, adst2T, A.Exp, scale=1.0 - NEG_ATT)
            ps_u82 = ps_bc.tile([128, R], f32, tag="bc")
            nc.tensor.matmul(ps_u82, ones16, u8row2, start=True, stop=True)
            nc.scalar.copy(u8bc2, ps_u82)

            # reuse L1 pools: o2 in ps_mm "h" shape, z2 rides an "agg" buffer
            ps_o2 = ps_mm.tile([128, R], f32, tag="h")
            ps_z2t = ps_ag.tile([HID + 1, R], f32, tag="agg")
            ps_z2 = ps_z2t[HID:HID + 1, :]
            for qt in range(QT):
                q4 = qpool.tile([128, 4, R], f16, tag="q")
                for k in range(4):
                    jt = qt * 4 + k
                    c8, r1 = jt // IT, jt % IT
                    nc.vector.tensor_scalar(
                        q4[:, k, :], u8bc2,
                        h2g_all[:, c8, r1, HID:HID + 1],
                        h2g_all[:, c8, r1, HID + 1:HID + 2],
                        op0=Al.mult, op1=Al.max)
                e4 = epool.tile([128, 4, R], f16, tag="e")
                nc.vector.tensor_tensor(
                    e4.rearrange("p a b -> p (a b)"),
                    q4.rearrange("p a b -> p (a b)"),
                    adjT_all[:, qt * 4:(qt + 1) * 4, :].rearrange(
                        "p a b -> p (a b)"),
                    op=Al.min)
                for k in range(4):
                    jt = qt * 4 + k
                    c8, r1 = jt // IT, jt % IT
                    nc.tensor.matmul(
                        ps_o2, h2g_all[:, c8, r1, 0:HID].bitcast(f16),
                        e4[:, k, :],
                        start=(jt == 0), stop=(jt == JT - 1))
                    nc.tensor.matmul(
                        ps_z2, ones_col16, e4[:, k, :],
                        start=(jt == 0), stop=(jt == JT - 1))
            # per-it: transpose z2 chunk -> recip col; transpose o2 -> prelu
            o2sb = work.tile([128, R], f32, tag="o2sb", bufs=1)
            for it in range(IT):
                nc.scalar.copy(o2sb[:, it * 128:(it + 1) * 128],
                               ps_o2[:, it * 128:(it + 1) * 128])
            z2sb = work.tile([1, R], f16, tag="z2sb", bufs=1)
            nc.vector.tensor_copy(z2sb, ps_z2)
            del ps_z2t
            outT_sb = work.tile([128, IT, OUT], f32, tag="outT", bufs=1)
            for it in range(IT):
                isl = slice(it * 128, (it + 1) * 128)
                ps_zTt = ps_sm.tile([128, 128], f32, tag="sm")
                ps_zT = ps_zTt.bitcast(f16)[:, 0:1]
                nc.tensor.transpose(ps_zT, z2sb[:, isl], ident16[0:1, 0:1])
                with nc.allow_low_precision(reason="1/z2 col fp16 src ok"):
                    nc.vector.reciprocal(rz2col[:, it:it + 1], ps_zT)
                ps_oT = ps_sm.tile([128, 128], f32, tag="sm")
                nc.tensor.transpose(ps_oT, o2sb[:, isl], ident)
                nc.scalar.activation(outT_sb[:, it, :], ps_oT, A.Prelu,
                                     scale=rz2col[:, it:it + 1], alpha=NEG_OUT)
            nc.sync.dma_start(
                out=d_out.rearrange("(i p) c -> p i c", p=128), in_=outT_sb)

    nc.finalize()
    return nc


def _prep_host(x, adj, w1, att_src1, att_dst1, b1, w2, att_src2, att_dst2, b2):
    x = np.asarray(x, np.float32).reshape(N, F_IN)
    adj = np.asarray(adj, np.float32).reshape(N, N)
    w1 = np.asarray(w1, np.float32)
    w2 = np.asarray(w2, np.float32)
    att_src1 = np.asarray(att_src1, np.float32)
    att_dst1 = np.asarray(att_dst1, np.float32)
    att_src2 = np.asarray(att_src2, np.float32)
    att_dst2 = np.asarray(att_dst2, np.float32)
    b1 = np.asarray(b1, np.float32)
    b2 = np.asarray(b2, np.float32)

    xT = np.ascontiguousarray(x.T)
    xT16 = xT.astype(np.float16)
    adjm = (adj * MASKV).astype(np.float16)
    v_src1 = np.empty((F_IN, H1), np.float32)
    v_dst1 = np.empty((F_IN, H1), np.float32)
    for h in range(H1):
        blk = w1[:, h * HID:(h + 1) * HID]
        v_src1[:, h] = blk @ att_src1[h]
        v_dst1[:, h] = blk @ att_dst1[h]
    rhs1 = np.ascontiguousarray(
        np.concatenate([w1, v_src1], axis=1)).astype(np.float16)
    v_src2 = (w2 @ att_src2[0])[:, None]
    v_dst2 = (w2 @ att_dst2[0])[:, None]
    rhs2 = np.ascontiguousarray(
        np.concatenate([w2, v_src2, v_dst2], axis=1)).astype(np.float16)
    b1c = np.ascontiguousarray(b1.reshape(H1, HID).T)
    b2r = np.zeros((1, OUT + 2), np.float16)
    b2r[0, 0:OUT] = b2

    in_maps = []
    for c in range(N_CORES):
        rows = slice(c * R, (c + 1) * R)
        in_maps.append({
            "xT16": xT16,
            "xmT": np.ascontiguousarray(xT16[:, rows]),
            "adjT": np.ascontiguousarray(adjm[rows, :].T),
            "rhs1": rhs1,
            "vdst1": v_dst1.astype(np.float16),
            "rhs2": rhs2,
            "b1c": b1c,
            "b2r": b2r,
        })
    return in_maps


def kernel(**inputs) -> np.ndarray:
    from concourse.bass_utils import run_bass_kernel_spmd

    if "nc" not in _CACHE:
        _CACHE["nc"] = _build()
    nc = _CACHE["nc"]
    in_maps = _prep_host(**inputs)
    try:
        res = run_bass_kernel_spmd(nc, in_maps, list(range(N_CORES)))
    except Exception:
        # transient NRT device wedge — one clean retry
        res = run_bass_kernel_spmd(nc, in_maps, list(range(N_CORES)))
    out = np.empty((1, N, OUT), np.float32)
    for c in range(N_CORES):
        out[0, c * R:(c + 1) * R, :] = res.results[c]["outR"]
    return out


# revision 39
# speedup vs baseline: 1.1033x; 1.0004x over previous
"""Dense GAT (2-layer, 8+1 heads) on 8 Trainium2 NeuronCores — V3.

Row-parallel over destination rows i (R=512 per core). Per core:
  - adjacency arrives HOST-TRANSPOSED as adjT[j, i] in {0, 65504} fp16
    (mask applied via tensor MIN, no PE transposes needed).
  - h1|a_src from one fp16 matmul chain against host-folded
    [w1 | w1@blockdiag(att_src1)]; fp16 PSUM.
  - scaled-attention trick: softmax over j is invariant to any per-i
    factor, so E is normalized by exp(0.2*ad_i):
        e~[j,i] = min(adjT[j,i], max(eas_j * u8[i], e2as_j))
    with u8 = exp(0.8*ad_i) broadcast (one per head), eas = exp(as_j),
    e2as = exp(0.2*as_j) per-partition scalars.
    DVE chain: one tensor_scalar (4x mode) + mask-min (quad-batched).
    ACT chain (some tiles): Relu(u8*eas - e2as) + Identity(r + e2as).
    Mask-min on DVE or GPSIMD (Pool) per static schedule.
  - softmax denominators ride as a ones column in the aggregation lhsT.
  - one AllGather of [512, 66] f32 (h2+b2 packed fp16 | eas2 | e2as2).
  - L2 output normalized via per-partition ACT scale after PE transpose.
"""
import numpy as np

N = 4096
F_IN = 256
HID = 64
H1 = 8
F1 = H1 * HID
OUT = 128
N_CORES = 8
R = N // N_CORES
JT = N // 128          # 32 j-tiles
IT = R // 128          # 4 i-tiles
QT = JT // 4           # 8 quads of 4 j-tiles
NEG_ATT = 0.2
NEG_OUT = 0.01
MASKV = 65504.0        # fp16 max: adjacency "1" value; mask via min()

G = HID + 2            # bounce cols: 64 f32 words (128 f16 h2) | eas2 | e2as2

_CACHE = {}

# ---- static engine schedule knobs ----
# ACT chain quads: (h, qt) pairs routed to the scalar engine (2 ACT ops/tile)
# per-head ACT-chain quad quota (ACT has h1-copy work early, idles late)
ACT_QUOTA = [1, 1, 2, 2, 2, 2, 3, 3]
_H1MOD = 2


def _is_act_quad(h, qt):
    return (qt * 5 + h) % 8 < ACT_QUOTA[h]


def _build():
    import concourse.bass as bass
    from concourse import bacc
    import concourse.mybir as mybir
    import concourse.tile as tile
    from concourse.masks import make_identity

    f32 = mybir.dt.float32
    f16 = mybir.dt.float16
    A = mybir.ActivationFunctionType
    Al = mybir.AluOpType

    nc = bacc.Bacc("TRN2", target_bir_lowering=False, debug=False,
                   num_devices=N_CORES)
    d_xT16 = nc.dram_tensor("xT16", [F_IN, N], f16, kind="ExternalInput")
    d_xmT = nc.dram_tensor("xmT", [F_IN, R], f16, kind="ExternalInput")
    d_adjT = nc.dram_tensor("adjT", [N, R], f16, kind="ExternalInput")
    d_rhs1 = nc.dram_tensor("rhs1", [F_IN, F1 + H1], f16, kind="ExternalInput")
    d_vdst1 = nc.dram_tensor("vdst1", [F_IN, H1], f16, kind="ExternalInput")
    d_rhs2 = nc.dram_tensor("rhs2", [F1, OUT + 2], f16, kind="ExternalInput")
    d_b1c = nc.dram_tensor("b1c", [HID, H1], f32, kind="ExternalInput")
    d_b2r = nc.dram_tensor("b2r", [1, OUT + 2], f16, kind="ExternalInput")
    d_out = nc.dram_tensor("outR", [R, OUT], f32, kind="ExternalOutput")

    with tile.TileContext(nc) as tc:
        with tc.tile_pool(name="const", bufs=1) as const, \
             tc.tile_pool(name="big", bufs=1) as big, \
             tc.tile_pool(name="work", bufs=3) as work, \
             tc.tile_pool(name="qpool", bufs=5) as qpool, \
             tc.tile_pool(name="epool", bufs=5) as epool, \
             tc.tile_pool(name="dram", bufs=1, space="DRAM") as dram, \
             tc.tile_pool(name="ps_mm", bufs=2, space="PSUM") as ps_mm, \
             tc.tile_pool(name="ps_bc", bufs=2, space="PSUM") as ps_bc, \
             tc.tile_pool(name="ps_ag", bufs=2, space="PSUM") as ps_ag, \
             tc.tile_pool(name="ps_sm", bufs=2, space="PSUM") as ps_sm:
            ident = const.tile([128, 128], f32)
            make_identity(nc, ident)
            ident16 = const.tile([128, 128], f16)
            nc.vector.tensor_copy(ident16, ident)
            ones16 = const.tile([1, 128], f16)
            nc.vector.memset(ones16, 1.0)
            ones_col16 = const.tile([128, 1], f16)
            nc.vector.memset(ones_col16, 1.0)
            rhs1_sb = const.tile([128, 2, F1 + H1], f16)
            nc.sync.dma_start(out=rhs1_sb[:, 0, :], in_=d_rhs1[0:128, :])
            nc.sync.dma_start(out=rhs1_sb[:, 1, :], in_=d_rhs1[128:256, :])
            vdst1_sb = const.tile([128, 2, H1], f16)
            nc.sync.dma_start(out=vdst1_sb[:, 0, :], in_=d_vdst1[0:128, :])
            nc.sync.dma_start(out=vdst1_sb[:, 1, :], in_=d_vdst1[128:256, :])
            rhs2_sb = const.tile([128, 4, OUT + 2], f16)
            for kt in range(4):
                nc.sync.dma_start(out=rhs2_sb[:, kt, :],
                                  in_=d_rhs2[kt * 128:(kt + 1) * 128, :])
            b1_sb = const.tile([HID, H1], f32)
            nc.sync.dma_start(out=b1_sb, in_=d_b1c[:, :])
            b2r_sb = const.tile([1, OUT + 2], f16)
            nc.sync.dma_start(out=b2r_sb, in_=d_b2r[:, :])

            # ---- big persistent arrays ----
            adjT_all = big.tile([128, JT, R], f16)       # 32 KB/part
            xT_sb = big.tile([128, 2, N], f16)           # 16 KB/part
            h1_all = big.tile([128, JT, H1, HID + 1], f16)  # 32.5 KB/part
            asrc16 = big.tile([128, JT, H1], f16)
            easrc = big.tile([128, JT, H1], f32)
            e2src = big.tile([128, JT, H1], f32)
            nege2 = big.tile([128, JT, H1], f32)
            adstT = big.tile([H1, R], f32)
            adst_rows = big.tile([1, H1, R], f32)
            adst2T = big.tile([1, R], f32)
            x2T_all = big.tile([128, 4, R], f16)
            u8bc = big.tile([128, 2, R], f16)            # 2-head pipeline
            h2g_all = big.tile([128, N_CORES, IT, G], f32)
            u8bc2 = big.tile([128, R], f16)
            rz2col = big.tile([128, IT], f32)

            nc.vector.memset(h1_all[:, :, :, HID:HID + 1], 1.0)

            # ---- input DMAs: xmT/xT first (a_dst + h1 need them), adjT after
            xmT_sb = big.tile([128, 2, R], f16)
            nc.sync.dma_start(out=xmT_sb[:, 0, :], in_=d_xmT[0:128, :])
            nc.sync.dma_start(out=xmT_sb[:, 1, :], in_=d_xmT[128:256, :])
            for c4 in range(4):
                csl = slice(c4 * (N // 4), (c4 + 1) * (N // 4))
                nc.sync.dma_start(out=xT_sb[:, 0, csl], in_=d_xT16[0:128, csl])
                nc.sync.dma_start(out=xT_sb[:, 1, csl],
                                  in_=d_xT16[128:256, csl])
            for jb in range(8):
                nc.sync.dma_start(
                    out=adjT_all[:, jb, :],
                    in_=d_adjT[jb * 128:(jb + 1) * 128, :])

            # ---- a_dst (own rows) -> adstT [H1, R] f32 ----
            for it in range(IT):
                ps_ad = ps_sm.tile([128, 128], f32, tag="sm")
                for kb in range(2):
                    nc.tensor.matmul(ps_ad[:, 0:H1],
                                     xmT_sb[:, kb, it * 128:(it + 1) * 128],
                                     vdst1_sb[:, kb, :],
                                     start=(kb == 0), stop=(kb == 1))
                adm = work.tile([128, H1], f32, tag="adm", bufs=2)
                nc.vector.tensor_copy(adm, ps_ad[:, 0:H1])
                ps_adT = ps_sm.tile([128, 128], f32, tag="sm")
                nc.tensor.transpose(ps_adT[0:H1, :], adm, ident)
                nc.vector.tensor_copy(adstT[:, it * 128:(it + 1) * 128],
                                      ps_adT[0:H1, :])
            for h in range(H1):
                nc.sync.dma_start(out=adst_rows[:, h, :], in_=adstT[h:h + 1, :])
            for jb in range(8, JT):
                nc.sync.dma_start(
                    out=adjT_all[:, jb, :],
                    in_=d_adjT[jb * 128:(jb + 1) * 128, :])

            # ---- h1 | a_src per jt ----
            for jt in range(JT):
                cols = slice(jt * 128, (jt + 1) * 128)
                ps_h = ps_mm.tile([128, F1], f32, tag="h")
                ps_ast = ps_sm.tile([128, 128], f32, tag="sm")
                ps_as = ps_ast[:, 0:H1]
                for kb in range(2):
                    nc.tensor.matmul(ps_h, xT_sb[:, kb, cols],
                                     rhs1_sb[:, kb, 0:F1],
                                     start=(kb == 0), stop=(kb == 1))
                    nc.tensor.matmul(ps_as, xT_sb[:, kb, cols],
                                     rhs1_sb[:, kb, F1:F1 + H1],
                                     start=(kb == 0), stop=(kb == 1))
                # h1 [j, h, c] copies: 1 in _H1MOD on DVE, rest on ACT
                if jt % _H1MOD == 0:
                    nc.vector.tensor_copy(
                        h1_all[:, jt, :, 0:HID],
                        ps_h.rearrange("p (h c) -> p h c", c=HID))
                else:
                    nc.scalar.copy(
                        h1_all[:, jt, :, 0:HID],
                        ps_h.rearrange("p (h c) -> p h c", c=HID))
                nc.scalar.copy(asrc16[:, jt, :], ps_as)
                if jt % 8 == 7:
                    gs = slice(jt - 7, jt + 1)
                    nc.scalar.activation(easrc[:, gs, :], asrc16[:, gs, :],
                                         A.Exp)
                    nc.scalar.activation(e2src[:, gs, :], asrc16[:, gs, :],
                                         A.Exp, scale=NEG_ATT)
                    nc.scalar.activation(nege2[:, gs, :], e2src[:, gs, :],
                                         A.Identity, scale=-1.0)

            # ---- layer-1 attention, head-pipelined ----
            def _pre_head(h):
                u8row = work.tile([1, R], f16, tag="u8row", bufs=2,
                                  name=f"u8r{h}")
                nc.scalar.activation(u8row, adst_rows[:, h, :], A.Exp,
                                     scale=1.0 - NEG_ATT)
                ps_u8 = ps_bc.tile([128, R], f32, tag="bc", name=f"psu8{h}")
                nc.tensor.matmul(ps_u8, ones16, u8row, start=True, stop=True)
                nc.scalar.copy(u8bc[:, h % 2, :], ps_u8)

            _pre_head(0)
            for h in range(H1):
                if h + 1 < H1:
                    _pre_head(h + 1)
                u8 = u8bc[:, h % 2, :]
                ps_agg = ps_ag.tile([HID + 1, R], f32, tag="agg")
                act_qts = [qt for qt in range(QT) if _is_act_quad(h, qt)]
                dve_qts = [qt for qt in range(QT) if not _is_act_quad(h, qt)]
                # ACT-chain q tiles first so the scalar engine runs ahead
                act_q4 = {}
                for qt in act_qts:
                    q4 = qpool.tile([128, 4, R], f16, tag="qa", bufs=3,
                                    name=f"qa{h}_{qt}")
                    for k in range(4):
                        jt = qt * 4 + k
                        r = work.tile([128, R], f16, tag="ract", bufs=3)
                        nc.scalar.activation(
                            r, u8, A.Relu,
                            bias=nege2[:, jt, h:h + 1],
                            scale=easrc[:, jt, h:h + 1])
                        nc.scalar.activation(
                            q4[:, k, :], r, A.Identity,
                            bias=e2src[:, jt, h:h + 1])
                    act_q4[qt] = q4
                n_mm = 0
                for qt in dve_qts + act_qts:
                    if qt in act_q4:
                        q4 = act_q4[qt]
                    else:
                        q4 = qpool.tile([128, 4, R], f16, tag="q")
                        for k in range(4):
                            jt = qt * 4 + k
                            nc.vector.tensor_scalar(
                                q4[:, k, :], u8,
                                easrc[:, jt, h:h + 1],
                                e2src[:, jt, h:h + 1],
                                op0=Al.mult, op1=Al.max)
                    e4 = epool.tile([128, 4, R], f16, tag="e")
                    nc.vector.tensor_tensor(
                        e4.rearrange("p a b -> p (a b)"),
                        q4.rearrange("p a b -> p (a b)"),
                        adjT_all[:, qt * 4:(qt + 1) * 4, :].rearrange(
                            "p a b -> p (a b)"),
                        op=Al.min)
                    for k in range(4):
                        jt = qt * 4 + k
                        nc.tensor.matmul(ps_agg, h1_all[:, jt, h, :],
                                         e4[:, k, :],
                                         start=(n_mm == 0),
                                         stop=(n_mm == JT - 1))
                        n_mm += 1
                rz = work.tile([1, R], f16, tag="rz", bufs=2)
                with nc.allow_low_precision(reason="1/z in fp16: 1e-3 rel ok"):
                    nc.vector.reciprocal(rz, ps_agg[HID:HID + 1, :])
                ps_rzb = ps_bc.tile([128, R], f32, tag="bc")
                nc.tensor.matmul(ps_rzb[0:HID, :], ones16[:, 0:HID], rz,
                                 start=True, stop=True)
                rzb_sb = work.tile([HID, R], f16, tag="rzb", bufs=2)
                nc.scalar.copy(rzb_sb, ps_rzb[0:HID, :])
                y_h = work.tile([HID, R], f16, tag="yh", bufs=2)
                nc.vector.tensor_mul(y_h, ps_agg[0:HID, :], rzb_sb)
                po = (h % 2) * HID
                nc.scalar.activation(
                    x2T_all[po:po + HID, h // 2, :], y_h, A.Prelu,
                    bias=b1_sb[:, h:h + 1], alpha=NEG_OUT)

            # ---- layer 2: h2 per it, bounce, single AllGather ----
            bounce_in = dram.tile([R, G], f32, name="bin")
            bounce_out = dram.tile([N_CORES, R, G], f32,
                                   addr_space="Shared", name="bout")
            for it in range(IT):
                ps_h2t = ps_mm.tile([128, R], f32, tag="h")
                ps_h2 = ps_h2t[:, 0:OUT + 2]
                for kt in range(4):
                    nc.tensor.matmul(
                        ps_h2,
                        x2T_all[:, kt, it * 128:(it + 1) * 128],
                        rhs2_sb[:, kt, :],
                        start=(kt == 0), stop=False)
                nc.tensor.matmul(ps_h2, ones16, b2r_sb,
                                 start=False, stop=True)
                h2m = work.tile([128, G], f32, tag="h2m", bufs=2)
                nc.scalar.copy(h2m[:, 0:HID].bitcast(f16),
                               ps_h2[:, 0:OUT])
                nc.scalar.activation(h2m[:, HID:HID + 1],
                                     ps_h2[:, OUT:OUT + 1], A.Exp)
                nc.scalar.activation(h2m[:, HID + 1:HID + 2],
                                     ps_h2[:, OUT:OUT + 1], A.Exp,
                                     scale=NEG_ATT)
                nc.sync.dma_start(
                    out=bounce_in[it * 128:(it + 1) * 128, :], in_=h2m)
                ad2m = work.tile([128, 1], f32, tag="ad2m", bufs=2)
                nc.scalar.copy(ad2m, ps_h2[:, OUT + 1:OUT + 2])
                ps_adT2 = ps_sm.tile([1, 128], f32, tag="sm")
                nc.tensor.transpose(ps_adT2, ad2m, ident)
                nc.scalar.copy(adst2T[:, it * 128:(it + 1) * 128],
                               ps_adT2)
            nc.gpsimd.collective_compute(
                "AllGather",
                bass.mybir.AluOpType.bypass,
                replica_groups=[list(range(N_CORES))],
                ins=[bounce_in.opt()],
                outs=[bounce_out.opt()],
            )
            for c8 in range(N_CORES):
                nc.sync.dma_start(
                    out=h2g_all[:, c8, :, :],
                    in_=bounce_out[c8].rearrange("(r1 p) g -> p r1 g", p=128))

            # ---- layer-2 attention ----
            u8row2 = work.tile([1, R], f16, tag="u8row", bufs=2)
            nc.scalar.activation(u8row2, adst2T, A.Exp, scale=1.0 - NEG_ATT)
            ps_u82 = ps_bc.tile([128, R], f32, tag="bc")
            nc.tensor.matmul(ps_u82, ones16, u8row2, start=True, stop=True)
            nc.scalar.copy(u8bc2, ps_u82)

            # reuse L1 pools: o2 in ps_mm "h" shape, z2 rides an "agg" buffer
            ps_o2 = ps_mm.tile([128, R], f32, tag="h")
            ps_z2t = ps_ag.tile([HID + 1, R], f32, tag="agg")
            ps_z2 = ps_z2t[HID:HID + 1, :]
            for qt in range(QT):
                q4 = qpool.tile([128, 4, R], f16, tag="q")
                for k in range(4):
                    jt = qt * 4 + k
                    c8, r1 = jt // IT, jt % IT
                    nc.vector.tensor_scalar(
                        q4[:, k, :], u8bc2,
                        h2g_all[:, c8, r1, HID:HID + 1],
                        h2g_all[:, c8, r1, HID + 1:HID + 2],
                        op0=Al.mult, op1=Al.max)
                e4 = epool.tile([128, 4, R], f16, tag="e")
                nc.vector.tensor_tensor(
                    e4.rearrange("p a b -> p (a b)"),
                    q4.rearrange("p a b -> p (a b)"),
                    adjT_all[:, qt * 4:(qt + 1) * 4, :].rearrange(
                        "p a b -> p (a b)"),
                    op=Al.min)
                for k in range(4):
                    jt = qt * 4 + k
                    c8, r1 = jt // IT, jt % IT
                    nc.tensor.matmul(
                        ps_o2, h2g_all[:, c8, r1, 0:HID].bitcast(f16),
                        e4[:, k, :],
                        start=(jt == 0), stop=(jt == JT - 1))
                    nc.tensor.matmul(
                        ps_z2, ones_col16, e4[:, k, :],
                        start=(jt == 0), stop=(jt == JT - 1))
            # per-it: transpose z2 chunk -> recip col; transpose o2 -> prelu
            o2sb = work.tile([128, R], f32, tag="o2sb", bufs=1)
            for it in range(IT):
                nc.scalar.copy(o2sb[:, it * 128:(it + 1) * 128],
                               ps_o2[:, it * 128:(it + 1) * 128])
            z2sb = work.tile([1, R], f16, tag="z2sb", bufs=1)
            nc.vector.tensor_copy(z2sb, ps_z2)
            del ps_z2t
            outT_sb = work.tile([128, IT, OUT], f32, tag="outT", bufs=1)
            for it in range(IT):
                isl = slice(it * 128, (it + 1) * 128)
                ps_zTt = ps_sm.tile([128, 128], f32, tag="sm")
                ps_zT = ps_zTt.bitcast(f16)[:, 0:1]
                nc.tensor.transpose(ps_zT, z2sb[:, isl], ident16[0:1, 0:1])
                with nc.allow_low_precision(reason="1/z2 col fp16 src ok"):
                    nc.vector.reciprocal(rz2col[:, it:it + 1], ps_zT)
                ps_oT = ps_sm.tile([128, 128], f32, tag="sm")
                nc.tensor.transpose(ps_oT, o2sb[:, isl], ident)
                nc.scalar.activation(outT_sb[:, it, :], ps_oT, A.Prelu,
                                     scale=rz2col[:, it:it + 1], alpha=NEG_OUT)
            nc.sync.dma_start(
                out=d_out.rearrange("(i p) c -> p i c", p=128), in_=outT_sb)

    nc.finalize()
    return nc


def _prep_host(x, adj, w1, att_src1, att_dst1, b1, w2, att_src2, att_dst2, b2):
    x = np.asarray(x, np.float32).reshape(N, F_IN)
    adj = np.asarray(adj, np.float32).reshape(N, N)
    w1 = np.asarray(w1, np.float32)
    w2 = np.asarray(w2, np.float32)
    att_src1 = np.asarray(att_src1, np.float32)
    att_dst1 = np.asarray(att_dst1, np.float32)
    att_src2 = np.asarray(att_src2, np.float32)
    att_dst2 = np.asarray(att_dst2, np.float32)
    b1 = np.asarray(b1, np.float32)
    b2 = np.asarray(b2, np.float32)

    xT = np.ascontiguousarray(x.T)
    xT16 = xT.astype(np.float16)
    adjm = (adj * MASKV).astype(np.float16)
    v_src1 = np.empty((F_IN, H1), np.float32)
    v_dst1 = np.empty((F_IN, H1), np.float32)
    for h in range(H1):
        blk = w1[:, h * HID:(h + 1) * HID]
        v_src1[:, h] = blk @ att_src1[h]
        v_dst1[:, h] = blk @ att_dst1[h]
    rhs1 = np.ascontiguousarray(
        np.concatenate([w1, v_src1], axis=1)).astype(np.float16)
    v_src2 = (w2 @ att_src2[0])[:, None]
    v_dst2 = (w2 @ att_dst2[0])[:, None]
    rhs2 = np.ascontiguousarray(
        np.concatenate([w2, v_src2, v_dst2], axis=1)).astype(np.float16)
    b1c = np.ascontiguousarray(b1.reshape(H1, HID).T)
    b2r = np.zeros((1, OUT + 2), np.float16)
    b2r[0, 0:OUT] = b2

    in_maps = []
    for c in range(N_CORES):
        rows = slice(c * R, (c + 1) * R)
        in_maps.append({
            "xT16": xT16,
            "xmT": np.ascontiguousarray(xT16[:, rows]),
            "adjT": np.ascontiguousarray(adjm[rows, :].T),
            "rhs1": rhs1,
            "vdst1": v_dst1.astype(np.float16),
            "rhs2": rhs2,
            "b1c": b1c,
            "b2r": b2r,
        })
    return in_maps


def kernel(**inputs) -> np.ndarray:
    from concourse.bass_utils import run_bass_kernel_spmd

    if "nc" not in _CACHE:
        _CACHE["nc"] = _build()
    nc = _CACHE["nc"]
    in_maps = _prep_host(**inputs)
    try:
        res = run_bass_kernel_spmd(nc, in_maps, list(range(N_CORES)))
    except Exception:
        # transient NRT device wedge — one clean retry
        res = run_bass_kernel_spmd(nc, in_maps, list(range(N_CORES)))
    out = np.empty((1, N, OUT), np.float32)
    for c in range(N_CORES):
        out[0, c * R:(c + 1) * R, :] = res.results[c]["outR"]
    return out


# revision 40
# speedup vs baseline: 1.1069x; 1.0033x over previous
"""Dense GAT (2-layer, 8+1 heads) on 8 Trainium2 NeuronCores — V3.

Row-parallel over destination rows i (R=512 per core). Per core:
  - adjacency arrives HOST-TRANSPOSED as adjT[j, i] in {0, 65504} fp16
    (mask applied via tensor MIN, no PE transposes needed).
  - h1|a_src from one fp16 matmul chain against host-folded
    [w1 | w1@blockdiag(att_src1)]; fp16 PSUM.
  - scaled-attention trick: softmax over j is invariant to any per-i
    factor, so E is normalized by exp(0.2*ad_i):
        e~[j,i] = min(adjT[j,i], max(eas_j * u8[i], e2as_j))
    with u8 = exp(0.8*ad_i) broadcast (one per head), eas = exp(as_j),
    e2as = exp(0.2*as_j) per-partition scalars.
    DVE chain: one tensor_scalar (4x mode) + mask-min (quad-batched).
    ACT chain (some tiles): Relu(u8*eas - e2as) + Identity(r + e2as).
    Mask-min on DVE or GPSIMD (Pool) per static schedule.
  - softmax denominators ride as a ones column in the aggregation lhsT.
  - one AllGather of [512, 66] f32 (h2+b2 packed fp16 | eas2 | e2as2).
  - L2 output normalized via per-partition ACT scale after PE transpose.
"""
import numpy as np

N = 4096
F_IN = 256
HID = 64
H1 = 8
F1 = H1 * HID
OUT = 128
N_CORES = 8
R = N // N_CORES
JT = N // 128          # 32 j-tiles
IT = R // 128          # 4 i-tiles
QT = JT // 4           # 8 quads of 4 j-tiles
NEG_ATT = 0.2
NEG_OUT = 0.01
MASKV = 65504.0        # fp16 max: adjacency "1" value; mask via min()

G = HID + 2            # bounce cols: 64 f32 words (128 f16 h2) | eas2 | e2as2

_CACHE = {}

# ---- static engine schedule knobs ----
# ACT chain quads: (h, qt) pairs routed to the scalar engine (2 ACT ops/tile)
# per-head ACT-chain quad quota (ACT has h1-copy work early, idles late)
ACT_QUOTA = [1, 2, 2, 2, 2, 2, 3, 3]
_H1MOD = 2


def _is_act_quad(h, qt):
    return (qt * 5 + h) % 8 < ACT_QUOTA[h]


def _build():
    import concourse.bass as bass
    from concourse import bacc
    import concourse.mybir as mybir
    import concourse.tile as tile
    from concourse.masks import make_identity

    f32 = mybir.dt.float32
    f16 = mybir.dt.float16
    A = mybir.ActivationFunctionType
    Al = mybir.AluOpType

    nc = bacc.Bacc("TRN2", target_bir_lowering=False, debug=False,
                   num_devices=N_CORES)
    d_xT16 = nc.dram_tensor("xT16", [F_IN, N], f16, kind="ExternalInput")
    d_xmT = nc.dram_tensor("xmT", [F_IN, R], f16, kind="ExternalInput")
    d_adjT = nc.dram_tensor("adjT", [N, R], f16, kind="ExternalInput")
    d_rhs1 = nc.dram_tensor("rhs1", [F_IN, F1 + H1], f16, kind="ExternalInput")
    d_vdst1 = nc.dram_tensor("vdst1", [F_IN, H1], f16, kind="ExternalInput")
    d_rhs2 = nc.dram_tensor("rhs2", [F1, OUT + 2], f16, kind="ExternalInput")
    d_b1c = nc.dram_tensor("b1c", [HID, H1], f32, kind="ExternalInput")
    d_b2r = nc.dram_tensor("b2r", [1, OUT + 2], f16, kind="ExternalInput")
    d_out = nc.dram_tensor("outR", [R, OUT], f32, kind="ExternalOutput")

    with tile.TileContext(nc) as tc:
        with tc.tile_pool(name="const", bufs=1) as const, \
             tc.tile_pool(name="big", bufs=1) as big, \
             tc.tile_pool(name="work", bufs=3) as work, \
             tc.tile_pool(name="qpool", bufs=5) as qpool, \
             tc.tile_pool(name="epool", bufs=5) as epool, \
             tc.tile_pool(name="dram", bufs=1, space="DRAM") as dram, \
             tc.tile_pool(name="ps_mm", bufs=2, space="PSUM") as ps_mm, \
             tc.tile_pool(name="ps_bc", bufs=2, space="PSUM") as ps_bc, \
             tc.tile_pool(name="ps_ag", bufs=2, space="PSUM") as ps_ag, \
             tc.tile_pool(name="ps_sm", bufs=2, space="PSUM") as ps_sm:
            ident = const.tile([128, 128], f32)
            make_identity(nc, ident)
            ident16 = const.tile([128, 128], f16)
            nc.vector.tensor_copy(ident16, ident)
            ones16 = const.tile([1, 128], f16)
            nc.vector.memset(ones16, 1.0)
            ones_col16 = const.tile([128, 1], f16)
            nc.vector.memset(ones_col16, 1.0)
            rhs1_sb = const.tile([128, 2, F1 + H1], f16)
            nc.sync.dma_start(out=rhs1_sb[:, 0, :], in_=d_rhs1[0:128, :])
            nc.sync.dma_start(out=rhs1_sb[:, 1, :], in_=d_rhs1[128:256, :])
            vdst1_sb = const.tile([128, 2, H1], f16)
            nc.sync.dma_start(out=vdst1_sb[:, 0, :], in_=d_vdst1[0:128, :])
            nc.sync.dma_start(out=vdst1_sb[:, 1, :], in_=d_vdst1[128:256, :])
            rhs2_sb = const.tile([128, 4, OUT + 2], f16)
            for kt in range(4):
                nc.sync.dma_start(out=rhs2_sb[:, kt, :],
                                  in_=d_rhs2[kt * 128:(kt + 1) * 128, :])
            b1_sb = const.tile([HID, H1], f32)
            nc.sync.dma_start(out=b1_sb, in_=d_b1c[:, :])
            b2r_sb = const.tile([1, OUT + 2], f16)
            nc.sync.dma_start(out=b2r_sb, in_=d_b2r[:, :])

            # ---- big persistent arrays ----
            adjT_all = big.tile([128, JT, R], f16)       # 32 KB/part
            xT_sb = big.tile([128, 2, N], f16)           # 16 KB/part
            h1_all = big.tile([128, JT, H1, HID + 1], f16)  # 32.5 KB/part
            asrc16 = big.tile([128, JT, H1], f16)
            easrc = big.tile([128, JT, H1], f32)
            e2src = big.tile([128, JT, H1], f32)
            nege2 = big.tile([128, JT, H1], f32)
            adstT = big.tile([H1, R], f32)
            adst_rows = big.tile([1, H1, R], f32)
            adst2T = big.tile([1, R], f32)
            x2T_all = big.tile([128, 4, R], f16)
            u8bc = big.tile([128, 2, R], f16)            # 2-head pipeline
            h2g_all = big.tile([128, N_CORES, IT, G], f32)
            u8bc2 = big.tile([128, R], f16)
            rz2col = big.tile([128, IT], f32)

            nc.vector.memset(h1_all[:, :, :, HID:HID + 1], 1.0)

            # ---- input DMAs: xmT/xT first (a_dst + h1 need them), adjT after
            xmT_sb = big.tile([128, 2, R], f16)
            nc.sync.dma_start(out=xmT_sb[:, 0, :], in_=d_xmT[0:128, :])
            nc.sync.dma_start(out=xmT_sb[:, 1, :], in_=d_xmT[128:256, :])
            for c4 in range(4):
                csl = slice(c4 * (N // 4), (c4 + 1) * (N // 4))
                nc.sync.dma_start(out=xT_sb[:, 0, csl], in_=d_xT16[0:128, csl])
                nc.sync.dma_start(out=xT_sb[:, 1, csl],
                                  in_=d_xT16[128:256, csl])
            for jb in range(8):
                nc.sync.dma_start(
                    out=adjT_all[:, jb, :],
                    in_=d_adjT[jb * 128:(jb + 1) * 128, :])

            # ---- a_dst (own rows) -> adstT [H1, R] f32 ----
            for it in range(IT):
                ps_ad = ps_sm.tile([128, 128], f32, tag="sm")
                for kb in range(2):
                    nc.tensor.matmul(ps_ad[:, 0:H1],
                                     xmT_sb[:, kb, it * 128:(it + 1) * 128],
                                     vdst1_sb[:, kb, :],
                                     start=(kb == 0), stop=(kb == 1))
                adm = work.tile([128, H1], f32, tag="adm", bufs=2)
                nc.vector.tensor_copy(adm, ps_ad[:, 0:H1])
                ps_adT = ps_sm.tile([128, 128], f32, tag="sm")
                nc.tensor.transpose(ps_adT[0:H1, :], adm, ident)
                nc.vector.tensor_copy(adstT[:, it * 128:(it + 1) * 128],
                                      ps_adT[0:H1, :])
            for h in range(H1):
                nc.sync.dma_start(out=adst_rows[:, h, :], in_=adstT[h:h + 1, :])
            for jb in range(8, JT):
                nc.sync.dma_start(
                    out=adjT_all[:, jb, :],
                    in_=d_adjT[jb * 128:(jb + 1) * 128, :])

            # ---- h1 | a_src per jt ----
            for jt in range(JT):
                cols = slice(jt * 128, (jt + 1) * 128)
                ps_h = ps_mm.tile([128, F1], f32, tag="h")
                ps_ast = ps_sm.tile([128, 128], f32, tag="sm")
                ps_as = ps_ast[:, 0:H1]
                for kb in range(2):
                    nc.tensor.matmul(ps_h, xT_sb[:, kb, cols],
                                     rhs1_sb[:, kb, 0:F1],
                                     start=(kb == 0), stop=(kb == 1))
                    nc.tensor.matmul(ps_as, xT_sb[:, kb, cols],
                                     rhs1_sb[:, kb, F1:F1 + H1],
                                     start=(kb == 0), stop=(kb == 1))
                # h1 [j, h, c] copies: 1 in _H1MOD on DVE, rest on ACT
                if jt % _H1MOD == 0:
                    nc.vector.tensor_copy(
                        h1_all[:, jt, :, 0:HID],
                        ps_h.rearrange("p (h c) -> p h c", c=HID))
                else:
                    nc.scalar.copy(
                        h1_all[:, jt, :, 0:HID],
                        ps_h.rearrange("p (h c) -> p h c", c=HID))
                nc.scalar.copy(asrc16[:, jt, :], ps_as)
                if jt % 8 == 7:
                    gs = slice(jt - 7, jt + 1)
                    nc.scalar.activation(easrc[:, gs, :], asrc16[:, gs, :],
                                         A.Exp)
                    nc.scalar.activation(e2src[:, gs, :], asrc16[:, gs, :],
                                         A.Exp, scale=NEG_ATT)
                    nc.scalar.activation(nege2[:, gs, :], e2src[:, gs, :],
                                         A.Identity, scale=-1.0)

            # ---- layer-1 attention, head-pipelined ----
            def _pre_head(h):
                u8row = work.tile([1, R], f16, tag="u8row", bufs=2,
                                  name=f"u8r{h}")
                nc.scalar.activation(u8row, adst_rows[:, h, :], A.Exp,
                                     scale=1.0 - NEG_ATT)
                ps_u8 = ps_bc.tile([128, R], f32, tag="bc", name=f"psu8{h}")
                nc.tensor.matmul(ps_u8, ones16, u8row, start=True, stop=True)
                nc.scalar.copy(u8bc[:, h % 2, :], ps_u8)

            _pre_head(0)
            for h in range(H1):
                if h + 1 < H1:
                    _pre_head(h + 1)
                u8 = u8bc[:, h % 2, :]
                ps_agg = ps_ag.tile([HID + 1, R], f32, tag="agg")
                act_qts = [qt for qt in range(QT) if _is_act_quad(h, qt)]
                dve_qts = [qt for qt in range(QT) if not _is_act_quad(h, qt)]
                # ACT-chain q tiles first so the scalar engine runs ahead
                act_q4 = {}
                for qt in act_qts:
                    q4 = qpool.tile([128, 4, R], f16, tag="qa", bufs=3,
                                    name=f"qa{h}_{qt}")
                    for k in range(4):
                        jt = qt * 4 + k
                        r = work.tile([128, R], f16, tag="ract", bufs=3)
                        nc.scalar.activation(
                            r, u8, A.Relu,
                            bias=nege2[:, jt, h:h + 1],
                            scale=easrc[:, jt, h:h + 1])
                        nc.scalar.activation(
                            q4[:, k, :], r, A.Identity,
                            bias=e2src[:, jt, h:h + 1])
                    act_q4[qt] = q4
                n_mm = 0
                for qt in dve_qts + act_qts:
                    if qt in act_q4:
                        q4 = act_q4[qt]
                    else:
                        q4 = qpool.tile([128, 4, R], f16, tag="q")
                        for k in range(4):
                            jt = qt * 4 + k
                            nc.vector.tensor_scalar(
                                q4[:, k, :], u8,
                                easrc[:, jt, h:h + 1],
                                e2src[:, jt, h:h + 1],
                                op0=Al.mult, op1=Al.max)
                    e4 = epool.tile([128, 4, R], f16, tag="e")
                    nc.vector.tensor_tensor(
                        e4.rearrange("p a b -> p (a b)"),
                        q4.rearrange("p a b -> p (a b)"),
                        adjT_all[:, qt * 4:(qt + 1) * 4, :].rearrange(
                            "p a b -> p (a b)"),
                        op=Al.min)
                    for k in range(4):
                        jt = qt * 4 + k
                        nc.tensor.matmul(ps_agg, h1_all[:, jt, h, :],
                                         e4[:, k, :],
                                         start=(n_mm == 0),
                                         stop=(n_mm == JT - 1))
                        n_mm += 1
                rz = work.tile([1, R], f16, tag="rz", bufs=2)
                with nc.allow_low_precision(reason="1/z in fp16: 1e-3 rel ok"):
                    nc.vector.reciprocal(rz, ps_agg[HID:HID + 1, :])
                ps_rzb = ps_bc.tile([128, R], f32, tag="bc")
                nc.tensor.matmul(ps_rzb[0:HID, :], ones16[:, 0:HID], rz,
                                 start=True, stop=True)
                rzb_sb = work.tile([HID, R], f16, tag="rzb", bufs=2)
                nc.scalar.copy(rzb_sb, ps_rzb[0:HID, :])
                y_h = work.tile([HID, R], f16, tag="yh", bufs=2)
                nc.vector.tensor_mul(y_h, ps_agg[0:HID, :], rzb_sb)
                po = (h % 2) * HID
                nc.scalar.activation(
                    x2T_all[po:po + HID, h // 2, :], y_h, A.Prelu,
                    bias=b1_sb[:, h:h + 1], alpha=NEG_OUT)

            # ---- layer 2: h2 per it, bounce, single AllGather ----
            bounce_in = dram.tile([R, G], f32, name="bin")
            bounce_out = dram.tile([N_CORES, R, G], f32,
                                   addr_space="Shared", name="bout")
            for it in range(IT):
                ps_h2t = ps_mm.tile([128, R], f32, tag="h")
                ps_h2 = ps_h2t[:, 0:OUT + 2]
                for kt in range(4):
                    nc.tensor.matmul(
                        ps_h2,
                        x2T_all[:, kt, it * 128:(it + 1) * 128],
                        rhs2_sb[:, kt, :],
                        start=(kt == 0), stop=False)
                nc.tensor.matmul(ps_h2, ones16, b2r_sb,
                                 start=False, stop=True)
                h2m = work.tile([128, G], f32, tag="h2m", bufs=2)
                nc.scalar.copy(h2m[:, 0:HID].bitcast(f16),
                               ps_h2[:, 0:OUT])
                nc.scalar.activation(h2m[:, HID:HID + 1],
                                     ps_h2[:, OUT:OUT + 1], A.Exp)
                nc.scalar.activation(h2m[:, HID + 1:HID + 2],
                                     ps_h2[:, OUT:OUT + 1], A.Exp,
                                     scale=NEG_ATT)
                nc.sync.dma_start(
                    out=bounce_in[it * 128:(it + 1) * 128, :], in_=h2m)
                ad2m = work.tile([128, 1], f32, tag="ad2m", bufs=2)
                nc.scalar.copy(ad2m, ps_h2[:, OUT + 1:OUT + 2])
                ps_adT2 = ps_sm.tile([1, 128], f32, tag="sm")
                nc.tensor.transpose(ps_adT2, ad2m, ident)
                nc.scalar.copy(adst2T[:, it * 128:(it + 1) * 128],
                               ps_adT2)
            nc.gpsimd.collective_compute(
                "AllGather",
                bass.mybir.AluOpType.bypass,
                replica_groups=[list(range(N_CORES))],
                ins=[bounce_in.opt()],
                outs=[bounce_out.opt()],
            )
            for c8 in range(N_CORES):
                nc.sync.dma_start(
                    out=h2g_all[:, c8, :, :],
                    in_=bounce_out[c8].rearrange("(r1 p) g -> p r1 g", p=128))

            # ---- layer-2 attention ----
            u8row2 = work.tile([1, R], f16, tag="u8row", bufs=2)
            nc.scalar.activation(u8row2, adst2T, A.Exp, scale=1.0 - NEG_ATT)
            ps_u82 = ps_bc.tile([128, R], f32, tag="bc")
            nc.tensor.matmul(ps_u82, ones16, u8row2, start=True, stop=True)
            nc.scalar.copy(u8bc2, ps_u82)

            # reuse L1 pools: o2 in ps_mm "h" shape, z2 rides an "agg" buffer
            ps_o2 = ps_mm.tile([128, R], f32, tag="h")
            ps_z2t = ps_ag.tile([HID + 1, R], f32, tag="agg")
            ps_z2 = ps_z2t[HID:HID + 1, :]
            for qt in range(QT):
                q4 = qpool.tile([128, 4, R], f16, tag="q")
                for k in range(4):
                    jt = qt * 4 + k
                    c8, r1 = jt // IT, jt % IT
                    nc.vector.tensor_scalar(
                        q4[:, k, :], u8bc2,
                        h2g_all[:, c8, r1, HID:HID + 1],
                        h2g_all[:, c8, r1, HID + 1:HID + 2],
                        op0=Al.mult, op1=Al.max)
                e4 = epool.tile([128, 4, R], f16, tag="e")
                nc.vector.tensor_tensor(
                    e4.rearrange("p a b -> p (a b)"),
                    q4.rearrange("p a b -> p (a b)"),
                    adjT_all[:, qt * 4:(qt + 1) * 4, :].rearrange(
                        "p a b -> p (a b)"),
                    op=Al.min)
                for k in range(4):
                    jt = qt * 4 + k
                    c8, r1 = jt // IT, jt % IT
                    nc.tensor.matmul(
                        ps_o2, h2g_all[:, c8, r1, 0:HID].bitcast(f16),
                        e4[:, k, :],
                        start=(jt == 0), stop=(jt == JT - 1))
                    nc.tensor.matmul(
                        ps_z2, ones_col16, e4[:, k, :],
                        start=(jt == 0), stop=(jt == JT - 1))
            # per-it: transpose z2 chunk -> recip col; transpose o2 -> prelu
            o2sb = work.tile([128, R], f32, tag="o2sb", bufs=1)
            for it in range(IT):
                nc.scalar.copy(o2sb[:, it * 128:(it + 1) * 128],
                               ps_o2[:, it * 128:(it + 1) * 128])
            z2sb = work.tile([1, R], f16, tag="z2sb", bufs=1)
            nc.vector.tensor_copy(z2sb, ps_z2)
            del ps_z2t
            outT_sb = work.tile([128, IT, OUT], f32, tag="outT", bufs=1)
            for it in range(IT):
                isl = slice(it * 128, (it + 1) * 128)
                ps_zTt = ps_sm.tile([128, 128], f32, tag="sm")
                ps_zT = ps_zTt.bitcast(f16)[:, 0:1]
                nc.tensor.transpose(ps_zT, z2sb[:, isl], ident16[0:1, 0:1])
                with nc.allow_low_precision(reason="1/z2 col fp16 src ok"):
                    nc.vector.reciprocal(rz2col[:, it:it + 1], ps_zT)
                ps_oT = ps_sm.tile([128, 128], f32, tag="sm")
                nc.tensor.transpose(ps_oT, o2sb[:, isl], ident)
                nc.scalar.activation(outT_sb[:, it, :], ps_oT, A.Prelu,
                                     scale=rz2col[:, it:it + 1], alpha=NEG_OUT)
            nc.sync.dma_start(
                out=d_out.rearrange("(i p) c -> p i c", p=128), in_=outT_sb)

    nc.finalize()
    return nc


def _prep_host(x, adj, w1, att_src1, att_dst1, b1, w2, att_src2, att_dst2, b2):
    x = np.asarray(x, np.float32).reshape(N, F_IN)
    adj = np.asarray(adj, np.float32).reshape(N, N)
    w1 = np.asarray(w1, np.float32)
    w2 = np.asarray(w2, np.float32)
    att_src1 = np.asarray(att_src1, np.float32)
    att_dst1 = np.asarray(att_dst1, np.float32)
    att_src2 = np.asarray(att_src2, np.float32)
    att_dst2 = np.asarray(att_dst2, np.float32)
    b1 = np.asarray(b1, np.float32)
    b2 = np.asarray(b2, np.float32)

    xT = np.ascontiguousarray(x.T)
    xT16 = xT.astype(np.float16)
    adjm = (adj * MASKV).astype(np.float16)
    v_src1 = np.empty((F_IN, H1), np.float32)
    v_dst1 = np.empty((F_IN, H1), np.float32)
    for h in range(H1):
        blk = w1[:, h * HID:(h + 1) * HID]
        v_src1[:, h] = blk @ att_src1[h]
        v_dst1[:, h] = blk @ att_dst1[h]
    rhs1 = np.ascontiguousarray(
        np.concatenate([w1, v_src1], axis=1)).astype(np.float16)
    v_src2 = (w2 @ att_src2[0])[:, None]
    v_dst2 = (w2 @ att_dst2[0])[:, None]
    rhs2 = np.ascontiguousarray(
        np.concatenate([w2, v_src2, v_dst2], axis=1)).astype(np.float16)
    b1c = np.ascontiguousarray(b1.reshape(H1, HID).T)
    b2r = np.zeros((1, OUT + 2), np.float16)
    b2r[0, 0:OUT] = b2

    in_maps = []
    for c in range(N_CORES):
        rows = slice(c * R, (c + 1) * R)
        in_maps.append({
            "xT16": xT16,
            "xmT": np.ascontiguousarray(xT16[:, rows]),
            "adjT": np.ascontiguousarray(adjm[rows, :].T),
            "rhs1": rhs1,
            "vdst1": v_dst1.astype(np.float16),
            "rhs2": rhs2,
            "b1c": b1c,
            "b2r": b2r,
        })
    return in_maps


def kernel(**inputs) -> np.ndarray:
    from concourse.bass_utils import run_bass_kernel_spmd

    if "nc" not in _CACHE:
        _CACHE["nc"] = _build()
    nc = _CACHE["nc"]
    in_maps = _prep_host(**inputs)
    try:
        res = run_bass_kernel_spmd(nc, in_maps, list(range(N_CORES)))
    except Exception:
        # transient NRT device wedge — one clean retry
        res = run_bass_kernel_spmd(nc, in_maps, list(range(N_CORES)))
    out = np.empty((1, N, OUT), np.float32)
    for c in range(N_CORES):
        out[0, c * R:(c + 1) * R, :] = res.results[c]["outR"]
    return out
